# revision 32
# baseline (speedup 1.0000x reference)
"""Trainium2 Bass kernel for a 12-head attention block with cls-token
rebalancing (B=4, N=1024, C=768), distributed over 8 NeuronCores.

Sharding: core = 2*b + g  (b = batch 0..3, g = head-group 0..1, 6 heads each).
Each core computes qkv / attention / softmax / cls-rebalance / attn@v for its
(batch, 6 heads), plus the partial output projection over its heads' channels;
core pairs {2b, 2b+1} AllReduce the projection partials.

Outputs: attn (4,12,1024,1024) fp32 and out (4,1024,768) fp32, matching
reference.py's (out, attn) tuple.
"""

import sys

if "/opt/trn_rl_repo" not in sys.path:
    sys.path.insert(0, "/opt/trn_rl_repo")

from contextlib import ExitStack

import numpy as np

import concourse.bass as bass
import concourse.bacc as bacc
import concourse.tile as tile
from concourse import mybir
from concourse import bass_utils

F32 = mybir.dt.float32
# TensorEngine compute dtype for the big matmuls: float32 (exact, 4 cyc/row)
# or float32r (fast path, 1 cyc/row at free-dim >= 256).
MM_DT = mybir.dt.float32

B, N, C = 4, 1024, 768
H = 12
HPC = 6  # heads per core
HD = 64
SCALE = HD ** -0.5
EPS = 1e-6
NCORES = 8
REPLICA_GROUPS = [[0, 1], [2, 3], [4, 5], [6, 7]]

Exp = mybir.ActivationFunctionType.Exp
ALU = mybir.AluOpType

# packed-input column offsets (all fp32, 128 partitions)
OFF_xT = 0        # 6 c_in tiles x 1024 n
OFF_wqkT = 6144   # 6 c_in tiles x 768 qk cols
OFF_wvT = 10752   # 6 c_in tiles x 384 v cols
OFF_wpjT = 13056  # 3 c_in tiles x 768 cols
OFF_ident = 15360
OFF_bqk = 15488   # (128, 6)
OFF_bvc = 15494   # (128, 3)
OFF_ones = 15497  # row (partition 0)
OFF_bv = 15625    # row
OFF_bpj = 16009   # row
OFF_cls = 16777   # row
PACKED = 16784


def _mm(ap):
    """Tag an fp32 AP with the matmul compute dtype."""
    if MM_DT is F32:
        return ap
    return ap.bitcast(MM_DT)


def build_bass():
    nc = bacc.Bacc("TRN2", debug=False, target_bir_lowering=False, num_devices=NCORES)

    # ---- external I/O: ONE packed input tensor (single DMA -> single
    # semaphore lane, since the PE LDWEIGHTS slot only fits one sync wait) ----
    inp_d = nc.dram_tensor("inp", (128, PACKED), F32, kind="ExternalInput").ap()

    attn_d = nc.dram_tensor("attn_out", (HPC, N, N), F32, kind="ExternalOutput").ap()
    out_d = nc.dram_tensor("out_ext", (N, C), F32, kind="ExternalOutput").ap()
    dbg_d = nc.dram_tensor("dbg", (16, N), F32, kind="ExternalOutput").ap()
    dbg2_d = nc.dram_tensor("dbg2", (128, 3 * N), F32, kind="ExternalOutput").ap()

    # ---- collective bounce buffers ----
    cc_in = nc.dram_tensor("cc_in", (N, C), F32).ap()
    cc_out = nc.dram_tensor("cc_out", (N, C), F32).ap()

    with tile.TileContext(nc) as tc, ExitStack() as ctx:
        P = ctx.enter_context  # pool opener

        persist = P(tc.tile_pool(name="persist", bufs=1))
        attn_pool = P(tc.tile_pool(name="attn", bufs=3))
        et_pool = P(tc.tile_pool(name="et", bufs=3))
        bc_pool = P(tc.tile_pool(name="bc", bufs=2))
        out_pool = P(tc.tile_pool(name="outsb", bufs=2))
        ps_pool = P(tc.tile_pool(name="ps", bufs=2, space="PSUM"))
        av_pool = P(tc.tile_pool(name="avps", bufs=2, space="PSUM"))

        # ---- persistent SBUF tensors ----
        inp = persist.tile([128, PACKED], F32, tag="inp")
        xT = inp[:, OFF_xT:OFF_xT + 6 * N]
        wqkT = inp[:, OFF_wqkT:OFF_wqkT + 6 * 768]
        wvT = inp[:, OFF_wvT:OFF_wvT + 6 * 384]
        wpjT = inp[:, OFF_wpjT:OFF_wpjT + 3 * C]
        ident = inp[:, OFF_ident:OFF_ident + 128]
        bqk = inp[:, OFF_bqk:OFF_bqk + 6]
        bvc = inp[:, OFF_bvc:OFF_bvc + 3]
        ones = inp[0:1, OFF_ones:OFF_ones + 128]
        bv = inp[0:1, OFF_bv:OFF_bv + HPC * HD]
        bpj = inp[0:1, OFF_bpj:OFF_bpj + C]
        clsb = inp[0:1, OFF_cls:OFF_cls + HPC]

        qkvT = persist.tile([128, 6 * N], F32, tag="qkvT")       # m 0..2: q pairs, 3..5: k pairs
        vsb = persist.tile([128, 8 * 384], F32, tag="vsb")       # 8 n tiles x (6 heads*64)
        outT = persist.tile([128, 3 * N], F32, tag="outT")       # pair j: c_in x n
        S = persist.tile([128, HPC * 8], F32, tag="S")           # row sums, head h cols h*8..
        iS = persist.tile([128, HPC * 8], F32, tag="iS")         # 1/S
        iSr = persist.tile([1, HPC * N], F32, tag="iSr")         # transposed 1/S rows
        v0T = persist.tile([128, 3], F32, tag="v0T")             # v[0,:] as columns
        cells = persist.tile([1, 8 * HPC], F32, tag="cells")     # per-head scalars

        # ---- one DMA for all inputs ----
        nc.sync.dma_start(out=inp[:, :], in_=inp_d[:, :])

        # ---- qk projection: qkvT[m] = (wqk tile m).T @ x, in [c_out, n] layout ----
        for m in range(6):
            ps = ps_pool.tile([128, N], F32, tag="ps")
            for nh in range(2):
                for k in range(6):
                    nc.tensor.matmul(
                        ps[:, nh * 512:(nh + 1) * 512],
                        _mm(wqkT[:, k * 768 + m * 128: k * 768 + (m + 1) * 128]),
                        _mm(xT[:, k * N + nh * 512: k * N + (nh + 1) * 512]),
                        start=(k == 0), stop=(k == 5),
                    )
            nc.vector.tensor_scalar_add(qkvT[:, m * N:(m + 1) * N], ps[:, :], bqk[:, m:m + 1])

        # ---- v projection in natural [n, d] layout: v = x.T.T @ wvT ----
        for nt in range(8):
            ps = ps_pool.tile([128, N], F32, tag="ps")
            for k in range(6):
                nc.tensor.matmul(
                    ps[:, 0:384],
                    _mm(xT[:, k * N + nt * 128: k * N + (nt + 1) * 128]),
                    _mm(wvT[:, k * 384:(k + 1) * 384]),
                    start=(k == 0), stop=False,
                )
            # + bias row via rank-1 ones matmul
            nc.tensor.matmul(ps[:, 0:384], _mm(ones[0:1, :]), _mm(bv[0:1, :]),
                             start=False, stop=True)
            nc.vector.tensor_copy(vsb[:, nt * 384:(nt + 1) * 384], ps[:, 0:384])

        # ---- v0T: v[0, :] as [d, 1] columns (for the cls row-0 output fix) ----
        for mt in range(3):
            ps = ps_pool.tile([128, N], F32, tag="ps")
            for k in range(6):
                nc.tensor.matmul(
                    ps[:, 0:1],
                    _mm(wvT[:, k * 384 + mt * 128: k * 384 + (mt + 1) * 128]),
                    _mm(xT[:, k * N: k * N + 1]),
                    start=(k == 0), stop=(k == 5),
                )
            nc.vector.tensor_scalar_add(v0T[:, mt:mt + 1], ps[:, 0:1], bvc[:, mt:mt + 1])

        # ---- attention per head-pair ----
        for j in range(3):
            qt_pair = qkvT[:, j * N:(j + 1) * N]
            kt_pair = qkvT[:, (3 + j) * N:(4 + j) * N]

            # pass 1: scores in [q, k] layout, softmax, cls-rebalance, attn out
            for half in range(2):
                h = 2 * j + half
                rows = slice(64 * half, 64 * half + 64)
                for qt in range(8):
                    ps = ps_pool.tile([128, N], F32, tag="ps")
                    for kh in range(2):
                        nc.tensor.matmul(
                            ps[:, kh * 512:(kh + 1) * 512],
                            _mm(qt_pair[rows, qt * 128:(qt + 1) * 128]),
                            _mm(kt_pair[rows, kh * 512:(kh + 1) * 512]),
                            start=True, stop=True,
                        )
                    at = attn_pool.tile([128, N], F32, tag="attn")
                    sc = S[:, h * 8 + qt: h * 8 + qt + 1]
                    nc.scalar.activation(at[:, :], ps[:, :], Exp, scale=SCALE, accum_out=sc)
                    isc = iS[:, h * 8 + qt: h * 8 + qt + 1]
                    nc.vector.reciprocal(isc, sc)
                    nc.vector.tensor_scalar_mul(at[:, :], at[:, :], isc)

                    if qt == 0:
                        # cls-token rebalancing on row 0 (partition 0 of tile 0)
                        cb = cells[0:1, h * 8: h * 8 + 8]
                        a00 = at[0:1, 0:1]
                        # c0 = a00_new = min(attn00 + cls_bias, 1)
                        nc.vector.tensor_scalar(cb[0:1, 0:1], a00, clsb[0:1, h:h + 1], 1.0,
                                                op0=ALU.add, op1=ALU.min)
                        # c1 = denom = (1 + eps) - attn00   (= actual + eps)
                        nc.vector.tensor_scalar(cb[0:1, 1:2], a00, -1.0, 1.0 + EPS,
                                                op0=ALU.mult, op1=ALU.add)
                        # c2 = 1/denom
                        nc.vector.reciprocal(cb[0:1, 2:3], cb[0:1, 1:2])
                        # c3 = 1 - a00_new
                        nc.vector.tensor_scalar(cb[0:1, 3:4], cb[0:1, 0:1], -1.0, 1.0,
                                                op0=ALU.mult, op1=ALU.add)
                        # c4 = mp = (1 - a00_new) / denom
                        nc.vector.tensor_mul(cb[0:1, 4:5], cb[0:1, 3:4], cb[0:1, 2:3])
                        # c5 = mp * attn00
                        nc.vector.tensor_mul(cb[0:1, 5:6], cb[0:1, 4:5], a00)
                        # c6 = beta = a00_new - mp * attn00
                        nc.vector.tensor_sub(cb[0:1, 6:7], cb[0:1, 0:1], cb[0:1, 5:6])
                        # rescale rest of row 0 by mp, then set attn[0,0] = a00_new
                        nc.vector.tensor_scalar_mul(at[0:1, 1:N], at[0:1, 1:N], cb[0:1, 4:5])
                        nc.vector.tensor_copy(at[0:1, 0:1], cb[0:1, 0:1])

                    nc.sync.dma_start(out=attn_d[h, qt * 128:(qt + 1) * 128, :], in_=at[:, :])

                # transpose invS for this head: 8x ([128,1] -> [1,128]) on row 0
                ps = ps_pool.tile([128, N], F32, tag="ps")
                for qt in range(8):
                    nc.tensor.transpose(ps[0:1, qt * 128:(qt + 1) * 128],
                                        iS[:, h * 8 + qt: h * 8 + qt + 1], ident[:, :])
                nc.vector.tensor_copy(iSr[0:1, h * N:(h + 1) * N], ps[0:1, :])

            # pass 2: scores.T in [k, q] layout, exp, attn@v (PE col-packed pair).
            # A and B halves accumulate in SEPARATE psum tiles: hardware allows
            # only one matmul accumulation group per psum bank at a time.
            avt = [av_pool.tile([128, N], F32, name=f"avt{_h}", tag="av") for _h in range(2)]
            for kt in range(8):
                for half in range(2):
                    h = 2 * j + half
                    rows = slice(64 * half, 64 * half + 64)
                    ps = ps_pool.tile([128, N], F32, tag="ps")
                    for qh in range(2):
                        nc.tensor.matmul(
                            ps[:, qh * 512:(qh + 1) * 512],
                            _mm(kt_pair[rows, kt * 128:(kt + 1) * 128]),
                            _mm(qt_pair[rows, qh * 512:(qh + 1) * 512]),
                            start=True, stop=True,
                        )
                    et = et_pool.tile([128, N], F32, tag="et")
                    nc.scalar.activation(et[:, :], ps[:, :], Exp, scale=SCALE)
                    vcol = vsb[:, kt * 384 + h * HD: kt * 384 + (h + 1) * HD]
                    for qh in range(2):
                        nc.tensor.matmul(
                            avt[half][rows, qh * 512:(qh + 1) * 512],
                            _mm(vcol),
                            _mm(et[:, qh * 512:(qh + 1) * 512]),
                            start=(kt == 0), stop=(kt == 7),
                            tile_position=(0, 64 * half),
                        )

            # normalize attn@v by 1/S[q] (free-dim scalar via partition bcast;
            # partition_broadcast writes from partition 0, so broadcast each
            # head's row to ALL partitions and slice the half we need)
            for c in range(8):
                for half in range(2):
                    h = 2 * j + half
                    rows = slice(64 * half, 64 * half + 64)
                    bc = bc_pool.tile([128, 128], F32, name=f"bc{half}", tag="bc")
                    nc.gpsimd.partition_broadcast(
                        bc[:, :], iSr[0:1, h * N + c * 128: h * N + (c + 1) * 128])
                    nc.vector.tensor_mul(
                        outT[rows, j * N + c * 128: j * N + (c + 1) * 128],
                        avt[half][rows, c * 128:(c + 1) * 128], bc[rows, :])

            # cls row-0 output fix: out[0,:] = mp*out[0,:] + beta*v[0,:]
            for half in range(2):
                h = 2 * j + half
                rows = slice(64 * half, 64 * half + 64)
                bc = bc_pool.tile([128, 128], F32, name=f"bcf{half}", tag="bc")
                nc.gpsimd.partition_broadcast(bc[:, 0:1], cells[0:1, h * 8 + 4: h * 8 + 5])
                nc.gpsimd.partition_broadcast(bc[:, 1:2], cells[0:1, h * 8 + 6: h * 8 + 7])
                v0 = v0T[rows, j: j + 1]
                col0 = outT[rows, j * N: j * N + 1]
                # bc[:,2] = beta * v0
                nc.vector.tensor_scalar_mul(bc[rows, 2:3], v0, bc[rows, 1:2])
                # col0 = mp * col0 + beta*v0
                nc.vector.scalar_tensor_tensor(col0, col0, bc[rows, 0:1], bc[rows, 2:3],
                                               op0=ALU.mult, op1=ALU.add)

        # ---- output projection partial: out_part[n, c] over this group's c_in ----
        for nt in range(8):
            ps = ps_pool.tile([128, N], F32, tag="ps")
            for ch in range(2):
                # bank-aligned regions: [0:384] in bank 0, [512:896] in bank 1
                cs = slice(ch * 512, ch * 512 + 384)
                for ktj in range(3):
                    nc.tensor.matmul(
                        ps[:, cs],
                        _mm(outT[:, ktj * N + nt * 128: ktj * N + (nt + 1) * 128]),
                        _mm(wpjT[:, ktj * C + ch * 384: ktj * C + (ch + 1) * 384]),
                        start=(ktj == 0), stop=False,
                    )
                nc.tensor.matmul(ps[:, cs], _mm(ones[0:1, :]),
                                 _mm(bpj[0:1, ch * 384:(ch + 1) * 384]),
                                 start=False, stop=True)
            ot = out_pool.tile([128, C], F32, tag="outsb")
            nc.vector.tensor_copy(ot[:, 0:384], ps[:, 0:384])
            nc.vector.tensor_copy(ot[:, 384:768], ps[:, 512:896])
            nc.sync.dma_start(out=cc_in[nt * 128:(nt + 1) * 128, :], in_=ot[:, :])
            if nt == 0:
                nc.sync.dma_start(out=dbg_d[9:10, 0:C], in_=ot[0:1, :])

        # ---- pair AllReduce of projection partials, then final out ----
        nc.gpsimd.collective_compute(
            "AllReduce", ALU.add, replica_groups=REPLICA_GROUPS,
            ins=[cc_in[:, :].opt()], outs=[cc_out[:, :].opt()],
        )
        nc.sync.dma_start(out=out_d[:, :], in_=cc_out[:, :])

        # debug dumps: iSr rows (6x1024), iS head0 (128x8 -> row 6), outT row (row 7)
        nc.sync.dma_start(out=dbg_d[0:6, :], in_=iSr[0:1, :].rearrange("p (h n) -> p h n", h=6))
        nc.sync.dma_start(out=dbg_d[6:7, 0:8], in_=iS[0:1, 0:8])
        nc.sync.dma_start(out=dbg_d[7:8, :], in_=outT[0:1, 0:N])
        nc.sync.dma_start(out=dbg_d[10:11, 0:C], in_=cc_in[0:1, :])
        nc.sync.dma_start(out=dbg_d[11:12, 0:C], in_=cc_out[0:1, :])
        nc.sync.dma_start(out=dbg2_d[:, :], in_=outT[:, :])

    nc.compile()
    _split_waits(nc)
    return nc


def _tiled_cols(a, kk):
    """(kk*128, M) -> (128, kk*M): column block k = rows k*128..(k+1)*128."""
    m = a.shape[1]
    return a.reshape(kk, 128, m).transpose(1, 0, 2).reshape(128, kk * m)


def _split_waits(nc):
    """Walrus codegen caps sync-waits at 1 per instruction (2 for
    EventSemaphore). Spill extra waits onto EventSemaphore NOPs inserted
    just before, on the same engine stream."""
    nid = [0]

    def nop_with(engine, waits):
        nid[0] += 1
        nop = mybir.InstEventSemaphore(name=f"WSPILL-{nid[0]}", ins=[], outs=[])
        nop.engine = engine
        nop.sync_info = mybir.SyncInfo(on_wait=list(waits), on_update=[])
        return nop

    for f in nc.m.functions:
        for blk in f.blocks:
            out = []
            changed = False
            for inst in blk.instructions:
                si = inst.sync_info
                waits = list(si.on_wait) if si is not None and si.on_wait else []
                cap = 2 if isinstance(inst, mybir.InstEventSemaphore) else 1
                if len(waits) > cap:
                    spill, keep = waits[:-cap], waits[-cap:]
                    for i in range(0, len(spill), 2):
                        out.append(nop_with(inst.engine, spill[i:i + 2]))
                    inst.sync_info = mybir.SyncInfo(
                        on_wait=keep, on_update=list(si.on_update) if si.on_update else [])
                    changed = True
                out.append(inst)
            if changed:
                blk.instructions = out


def make_in_maps(x, qkv_w, qkv_b, proj_w, proj_b, cls_bias):
    f = np.float32
    in_maps = []
    for core in range(NCORES):
        b, g = core // 2, core % 2
        hs = g * HPC
        qrows = qkv_w[hs * HD:(hs + HPC) * HD]            # (384, 768)
        krows = qkv_w[C + hs * HD: C + (hs + HPC) * HD]   # (384, 768)
        vrows = qkv_w[2 * C + hs * HD: 2 * C + (hs + HPC) * HD]
        bq = qkv_b[hs * HD:(hs + HPC) * HD]
        bk = qkv_b[C + hs * HD: C + (hs + HPC) * HD]
        bvv = qkv_b[2 * C + hs * HD: 2 * C + (hs + HPC) * HD]

        packed = np.zeros((128, PACKED), f)
        packed[:, OFF_xT:OFF_xT + 6 * N] = _tiled_cols(np.asarray(x[b]).T.astype(f), 6)
        packed[:, OFF_wqkT:OFF_wqkT + 6 * 768] = _tiled_cols(
            np.concatenate([qrows, krows], 0).T.astype(f), 6)
        packed[:, OFF_wvT:OFF_wvT + 6 * 384] = _tiled_cols(vrows.T.astype(f), 6)
        packed[:, OFF_wpjT:OFF_wpjT + 3 * C] = _tiled_cols(
            np.asarray(proj_w).T[hs * HD:(hs + HPC) * HD, :].astype(f), 3)
        packed[:, OFF_ident:OFF_ident + 128] = np.eye(128, dtype=f)
        packed[:, OFF_bqk:OFF_bqk + 6] = np.concatenate([bq, bk]).reshape(6, 128).T
        packed[:, OFF_bvc:OFF_bvc + 3] = np.asarray(bvv).reshape(3, 128).T
        packed[0, OFF_ones:OFF_ones + 128] = 1.0
        packed[0, OFF_bv:OFF_bv + HPC * HD] = bvv
        packed[0, OFF_bpj:OFF_bpj + C] = np.asarray(proj_b) * 0.5
        packed[0, OFF_cls:OFF_cls + HPC] = cls_bias[hs:hs + HPC]
        in_maps.append({"inp": packed})
    return in_maps


_CACHED_NC = None


def _get_nc():
    global _CACHED_NC
    if _CACHED_NC is None:
        _CACHED_NC = build_bass()
    return _CACHED_NC


def run(trace=False, **inputs):
    nc = _get_nc()
    in_maps = make_in_maps(**inputs)
    res = bass_utils.run_bass_kernel_spmd(
        nc, in_maps, core_ids=list(range(NCORES)), trace=trace,
    )
    attn = np.empty((B, H, N, N), np.float32)
    out = np.empty((B, N, C), np.float32)
    for core in range(NCORES):
        b, g = core // 2, core % 2
        attn[b, g * HPC:(g + 1) * HPC] = res.results[core]["attn_out"]
        if g == 0:
            out[b] = res.results[core]["out_ext"]
    return (out, attn), res


def kernel(**inputs):
    outputs, _ = run(trace=False, **inputs)
    return outputs


# revision 38
# speedup vs baseline: 1.4653x; 1.4653x over previous
"""Trainium2 Bass kernel for a 12-head attention block with cls-token
rebalancing (B=4, N=1024, C=768), distributed over 8 NeuronCores.

Sharding: core = 2*b + g  (b = batch 0..3, g = head-group 0..1, 6 heads each).
Each core computes qkv / attention / softmax / cls-rebalance / attn@v for its
(batch, 6 heads), plus the partial output projection over its heads' channels;
core pairs {2b, 2b+1} AllReduce the projection partials.

Outputs: attn (4,12,1024,1024) fp32 and out (4,1024,768) fp32, matching
reference.py's (out, attn) tuple.
"""

import sys

if "/opt/trn_rl_repo" not in sys.path:
    sys.path.insert(0, "/opt/trn_rl_repo")

from contextlib import ExitStack

import numpy as np

import concourse.bass as bass
import concourse.bacc as bacc
import concourse.tile as tile
from concourse import mybir
from concourse import bass_utils

F32 = mybir.dt.float32
# TensorEngine compute dtype for the big matmuls: float32r is the fast PE path
# (1 cyc/row at free-dim >= 256 vs 4 for float32). Tensors feeding matmuls are
# tagged float32r natively so the BIR verifier sees rounded producers.
R32 = mybir.dt.float32r

B, N, C = 4, 1024, 768
H = 12
HPC = 6  # heads per core
HD = 64
SCALE = HD ** -0.5
EPS = 1e-6
NCORES = 8
REPLICA_GROUPS = [[0, 1], [2, 3], [4, 5], [6, 7]]

Exp = mybir.ActivationFunctionType.Exp
ALU = mybir.AluOpType

# packed-input column offsets (all fp32, 128 partitions)
OFF_xT = 0        # 6 c_in tiles x 1024 n
OFF_wqkT = 6144   # 6 c_in tiles x 768 qk cols
OFF_wvT = 10752   # 6 c_in tiles x 384 v cols
OFF_wpjT = 13056  # 3 c_in tiles x 768 cols
OFF_ident = 15360
OFF_bqk = 15488   # (128, 6)
OFF_bvc = 15494   # (128, 3)
OFF_ones = 15497  # row (partition 0)
OFF_bv = 15625    # row
OFF_bpj = 16009   # row
OFF_cls = 16777   # row
PACKED = 16784


def _mm(ap):
    return ap


def build_bass():
    nc = bacc.Bacc("TRN2", debug=False, target_bir_lowering=False, num_devices=NCORES)

    # ---- external I/O: ONE packed input tensor (single DMA -> single
    # semaphore lane, since the PE LDWEIGHTS slot only fits one sync wait) ----
    inp_d = nc.dram_tensor("inp", (128, PACKED), R32, kind="ExternalInput").ap()

    attn_d = nc.dram_tensor("attn_out", (HPC, N, N), F32, kind="ExternalOutput").ap()
    out_d = nc.dram_tensor("out_ext", (N, C), F32, kind="ExternalOutput").ap()
    dbg_d = None  # debug outputs disabled

    # ---- collective bounce buffers ----
    cc_in = nc.dram_tensor("cc_in", (N, C), F32).ap()
    cc_out = nc.dram_tensor("cc_out", (N, C), F32).ap()

    with tile.TileContext(nc) as tc, ExitStack() as ctx:
        P = ctx.enter_context  # pool opener

        persist = P(tc.tile_pool(name="persist", bufs=1))
        attn_pool = P(tc.tile_pool(name="attn", bufs=3))
        et_pool = P(tc.tile_pool(name="et", bufs=3))
        bc_pool = P(tc.tile_pool(name="bc", bufs=2))
        out_pool = P(tc.tile_pool(name="outsb", bufs=2))
        ps_pool = P(tc.tile_pool(name="ps", bufs=2, space="PSUM"))
        av_pool = P(tc.tile_pool(name="avps", bufs=2, space="PSUM"))

        # ---- persistent SBUF tensors ----
        inp = persist.tile([128, PACKED], R32, tag="inp")
        xT = inp[:, OFF_xT:OFF_xT + 6 * N]
        wqkT = inp[:, OFF_wqkT:OFF_wqkT + 6 * 768]
        wvT = inp[:, OFF_wvT:OFF_wvT + 6 * 384]
        wpjT = inp[:, OFF_wpjT:OFF_wpjT + 3 * C]
        ident = inp[:, OFF_ident:OFF_ident + 128].bitcast(F32)
        bqk = inp[:, OFF_bqk:OFF_bqk + 6].bitcast(F32)
        bvc = inp[:, OFF_bvc:OFF_bvc + 3].bitcast(F32)
        ones = inp[0:1, OFF_ones:OFF_ones + 128]
        bv = inp[0:1, OFF_bv:OFF_bv + HPC * HD]
        bpj = inp[0:1, OFF_bpj:OFF_bpj + C]
        clsb = inp[0:1, OFF_cls:OFF_cls + HPC].bitcast(F32)

        qkvT = persist.tile([128, 6 * N], R32, tag="qkvT")       # m 0..2: q pairs, 3..5: k pairs
        vsb = persist.tile([128, 8 * 384], R32, tag="vsb")       # 8 n tiles x (6 heads*64)
        outT = persist.tile([128, 3 * N], R32, tag="outT")       # pair j: c_in x n
        S = persist.tile([128, HPC * 8], F32, tag="S")           # row sums, head h cols h*8..
        iS = persist.tile([128, HPC * 8], F32, tag="iS")         # 1/S
        iSr = persist.tile([1, HPC * N], F32, tag="iSr")         # transposed 1/S rows
        v0T = persist.tile([128, 3], F32, tag="v0T")             # v[0,:] as columns
        cells = persist.tile([1, 8 * HPC], F32, tag="cells")     # per-head scalars

        # ---- one DMA for all inputs ----
        nc.sync.dma_start(out=inp[:, :], in_=inp_d[:, :])

        # ---- qk projection: qkvT[m] = (wqk tile m).T @ x, in [c_out, n] layout ----
        for m in range(6):
            ps = ps_pool.tile([128, N], F32, tag="ps")
            for nh in range(2):
                for k in range(6):
                    nc.tensor.matmul(
                        ps[:, nh * 512:(nh + 1) * 512],
                        _mm(wqkT[:, k * 768 + m * 128: k * 768 + (m + 1) * 128]),
                        _mm(xT[:, k * N + nh * 512: k * N + (nh + 1) * 512]),
                        start=(k == 0), stop=(k == 5),
                    )
            nc.vector.tensor_scalar_add(qkvT[:, m * N:(m + 1) * N], ps[:, :], bqk[:, m:m + 1])

        # ---- v projection in natural [n, d] layout: v = x.T.T @ wvT ----
        for nt in range(8):
            ps = ps_pool.tile([128, N], F32, tag="ps")
            for k in range(6):
                nc.tensor.matmul(
                    ps[:, 0:384],
                    _mm(xT[:, k * N + nt * 128: k * N + (nt + 1) * 128]),
                    _mm(wvT[:, k * 384:(k + 1) * 384]),
                    start=(k == 0), stop=False,
                )
            # + bias row via rank-1 ones matmul
            nc.tensor.matmul(ps[:, 0:384], _mm(ones[0:1, :]), _mm(bv[0:1, :]),
                             start=False, stop=True)
            nc.vector.tensor_copy(vsb[:, nt * 384:(nt + 1) * 384], ps[:, 0:384])

        # ---- v0T: v[0, :] as [d, 1] columns (for the cls row-0 output fix) ----
        for mt in range(3):
            ps = ps_pool.tile([128, N], F32, tag="ps")
            for k in range(6):
                nc.tensor.matmul(
                    ps[:, 0:1],
                    wvT[:, k * 384 + mt * 128: k * 384 + (mt + 1) * 128].bitcast(F32),
                    xT[:, k * N: k * N + 1].bitcast(F32),
                    start=(k == 0), stop=(k == 5),
                )
            nc.vector.tensor_scalar_add(v0T[:, mt:mt + 1], ps[:, 0:1], bvc[:, mt:mt + 1])

        # ---- attention per head-pair ----
        for j in range(3):
            qt_pair = qkvT[:, j * N:(j + 1) * N]
            kt_pair = qkvT[:, (3 + j) * N:(4 + j) * N]

            # pass 1: scores in [q, k] layout, softmax, cls-rebalance, attn out
            for half in range(2):
                h = 2 * j + half
                rows = slice(64 * half, 64 * half + 64)
                for qt in range(8):
                    ps = ps_pool.tile([128, N], F32, tag="ps")
                    for kh in range(2):
                        nc.tensor.matmul(
                            ps[:, kh * 512:(kh + 1) * 512],
                            _mm(qt_pair[rows, qt * 128:(qt + 1) * 128]),
                            _mm(kt_pair[rows, kh * 512:(kh + 1) * 512]),
                            start=True, stop=True,
                        )
                    at = attn_pool.tile([128, N], F32, tag="attn")
                    sc = S[:, h * 8 + qt: h * 8 + qt + 1]
                    nc.scalar.activation(at[:, :], ps[:, :], Exp, scale=SCALE, accum_out=sc)
                    isc = iS[:, h * 8 + qt: h * 8 + qt + 1]
                    nc.vector.reciprocal(isc, sc)
                    nc.vector.tensor_scalar_mul(at[:, :], at[:, :], isc)

                    if qt == 0:
                        # cls-token rebalancing on row 0 (partition 0 of tile 0)
                        cb = cells[0:1, h * 8: h * 8 + 8]
                        a00 = at[0:1, 0:1]
                        # c0 = a00_new = min(attn00 + cls_bias, 1)
                        nc.vector.tensor_scalar(cb[0:1, 0:1], a00, clsb[0:1, h:h + 1], 1.0,
                                                op0=ALU.add, op1=ALU.min)
                        # c1 = denom = (1 + eps) - attn00   (= actual + eps)
                        nc.vector.tensor_scalar(cb[0:1, 1:2], a00, -1.0, 1.0 + EPS,
                                                op0=ALU.mult, op1=ALU.add)
                        # c2 = 1/denom
                        nc.vector.reciprocal(cb[0:1, 2:3], cb[0:1, 1:2])
                        # c3 = 1 - a00_new
                        nc.vector.tensor_scalar(cb[0:1, 3:4], cb[0:1, 0:1], -1.0, 1.0,
                                                op0=ALU.mult, op1=ALU.add)
                        # c4 = mp = (1 - a00_new) / denom
                        nc.vector.tensor_mul(cb[0:1, 4:5], cb[0:1, 3:4], cb[0:1, 2:3])
                        # c5 = mp * attn00
                        nc.vector.tensor_mul(cb[0:1, 5:6], cb[0:1, 4:5], a00)
                        # c6 = beta = a00_new - mp * attn00
                        nc.vector.tensor_sub(cb[0:1, 6:7], cb[0:1, 0:1], cb[0:1, 5:6])
                        # rescale rest of row 0 by mp, then set attn[0,0] = a00_new
                        nc.vector.tensor_scalar_mul(at[0:1, 1:N], at[0:1, 1:N], cb[0:1, 4:5])
                        nc.vector.tensor_copy(at[0:1, 0:1], cb[0:1, 0:1])

                    nc.sync.dma_start(out=attn_d[h, qt * 128:(qt + 1) * 128, :], in_=at[:, :])

                # transpose invS for this head: 8x ([128,1] -> [1,128]) on row 0
                ps = ps_pool.tile([128, N], F32, tag="ps")
                for qt in range(8):
                    nc.tensor.transpose(ps[0:1, qt * 128:(qt + 1) * 128],
                                        iS[:, h * 8 + qt: h * 8 + qt + 1], ident[:, :])
                nc.vector.tensor_copy(iSr[0:1, h * N:(h + 1) * N], ps[0:1, :])

            # pass 2: scores.T in [k, q] layout, exp, attn@v (PE col-packed pair).
            # A and B halves accumulate in SEPARATE psum tiles: hardware allows
            # only one matmul accumulation group per psum bank at a time.
            avt = [av_pool.tile([128, N], F32, name=f"avt{_h}", tag="av") for _h in range(2)]
            for kt in range(8):
                for half in range(2):
                    h = 2 * j + half
                    rows = slice(64 * half, 64 * half + 64)
                    ps = ps_pool.tile([128, N], F32, tag="ps")
                    for qh in range(2):
                        nc.tensor.matmul(
                            ps[:, qh * 512:(qh + 1) * 512],
                            _mm(kt_pair[rows, kt * 128:(kt + 1) * 128]),
                            _mm(qt_pair[rows, qh * 512:(qh + 1) * 512]),
                            start=True, stop=True,
                        )
                    et = et_pool.tile([128, N], R32, tag="et")
                    nc.scalar.activation(et[:, :], ps[:, :], Exp, scale=SCALE)
                    # stationary operand = both heads' v columns (128 wide);
                    # only this half's 64 output rows are meaningful, the other
                    # 64 rows of avt[half] are never read.
                    vcol = vsb[:, kt * 384 + j * 128: kt * 384 + (j + 1) * 128]
                    for qh in range(2):
                        nc.tensor.matmul(
                            avt[half][:, qh * 512:(qh + 1) * 512],
                            _mm(vcol),
                            _mm(et[:, qh * 512:(qh + 1) * 512]),
                            start=(kt == 0), stop=(kt == 7),
                        )

            # normalize attn@v by 1/S[q] (free-dim scalar via partition bcast;
            # partition_broadcast writes from partition 0, so broadcast each
            # head's row to ALL partitions and slice the half we need)
            for c in range(8):
                for half in range(2):
                    h = 2 * j + half
                    rows = slice(64 * half, 64 * half + 64)
                    bc = bc_pool.tile([128, 128], F32, name=f"bc{half}", tag="bc")
                    nc.gpsimd.partition_broadcast(
                        bc[:, :], iSr[0:1, h * N + c * 128: h * N + (c + 1) * 128])
                    nc.vector.tensor_mul(
                        outT[rows, j * N + c * 128: j * N + (c + 1) * 128],
                        avt[half][rows, c * 128:(c + 1) * 128], bc[rows, :])

            # cls row-0 output fix: out[0,:] = mp*out[0,:] + beta*v[0,:]
            for half in range(2):
                h = 2 * j + half
                rows = slice(64 * half, 64 * half + 64)
                bc = bc_pool.tile([128, 128], F32, name=f"bcf{half}", tag="bc")
                nc.gpsimd.partition_broadcast(bc[:, 0:1], cells[0:1, h * 8 + 4: h * 8 + 5])
                nc.gpsimd.partition_broadcast(bc[:, 1:2], cells[0:1, h * 8 + 6: h * 8 + 7])
                v0 = v0T[rows, j: j + 1]
                col0 = outT[rows, j * N: j * N + 1]
                # bc[:,2] = beta * v0
                nc.vector.tensor_scalar_mul(bc[rows, 2:3], v0, bc[rows, 1:2])
                # col0 = mp * col0 + beta*v0
                nc.vector.scalar_tensor_tensor(col0, col0, bc[rows, 0:1], bc[rows, 2:3],
                                               op0=ALU.mult, op1=ALU.add)

        # ---- output projection partial: out_part[n, c] over this group's c_in ----
        for nt in range(8):
            ps = ps_pool.tile([128, N], F32, tag="ps")
            for ch in range(2):
                # bank-aligned regions: [0:384] in bank 0, [512:896] in bank 1
                cs = slice(ch * 512, ch * 512 + 384)
                for ktj in range(3):
                    nc.tensor.matmul(
                        ps[:, cs],
                        _mm(outT[:, ktj * N + nt * 128: ktj * N + (nt + 1) * 128]),
                        _mm(wpjT[:, ktj * C + ch * 384: ktj * C + (ch + 1) * 384]),
                        start=(ktj == 0), stop=False,
                    )
                nc.tensor.matmul(ps[:, cs], _mm(ones[0:1, :]),
                                 _mm(bpj[0:1, ch * 384:(ch + 1) * 384]),
                                 start=False, stop=True)
            ot = out_pool.tile([128, C], F32, tag="outsb")
            nc.vector.tensor_copy(ot[:, 0:384], ps[:, 0:384])
            nc.vector.tensor_copy(ot[:, 384:768], ps[:, 512:896])
            nc.sync.dma_start(out=cc_in[nt * 128:(nt + 1) * 128, :], in_=ot[:, :])

        # ---- pair AllReduce of projection partials, then final out ----
        nc.gpsimd.collective_compute(
            "AllReduce", ALU.add, replica_groups=REPLICA_GROUPS,
            ins=[cc_in[:, :].opt()], outs=[cc_out[:, :].opt()],
        )
        nc.sync.dma_start(out=out_d[:, :], in_=cc_out[:, :])


    nc.compile()
    _split_waits(nc)
    return nc


def _tiled_cols(a, kk):
    """(kk*128, M) -> (128, kk*M): column block k = rows k*128..(k+1)*128."""
    m = a.shape[1]
    return a.reshape(kk, 128, m).transpose(1, 0, 2).reshape(128, kk * m)


def _split_waits(nc):
    """Walrus codegen caps sync-waits at 1 per instruction (2 for
    EventSemaphore). Spill extra waits onto EventSemaphore NOPs inserted
    just before, on the same engine stream."""
    nid = [0]

    def nop_with(engine, waits):
        nid[0] += 1
        nop = mybir.InstEventSemaphore(name=f"WSPILL-{nid[0]}", ins=[], outs=[])
        nop.engine = engine
        nop.sync_info = mybir.SyncInfo(on_wait=list(waits), on_update=[])
        return nop

    for f in nc.m.functions:
        for blk in f.blocks:
            out = []
            changed = False
            for inst in blk.instructions:
                si = inst.sync_info
                waits = list(si.on_wait) if si is not None and si.on_wait else []
                cap = 2 if isinstance(inst, mybir.InstEventSemaphore) else 1
                if len(waits) > cap:
                    spill, keep = waits[:-cap], waits[-cap:]
                    for i in range(0, len(spill), 2):
                        out.append(nop_with(inst.engine, spill[i:i + 2]))
                    inst.sync_info = mybir.SyncInfo(
                        on_wait=keep, on_update=list(si.on_update) if si.on_update else [])
                    changed = True
                out.append(inst)
            if changed:
                blk.instructions = out


def make_in_maps(x, qkv_w, qkv_b, proj_w, proj_b, cls_bias):
    f = np.float32
    in_maps = []
    for core in range(NCORES):
        b, g = core // 2, core % 2
        hs = g * HPC
        qrows = qkv_w[hs * HD:(hs + HPC) * HD]            # (384, 768)
        krows = qkv_w[C + hs * HD: C + (hs + HPC) * HD]   # (384, 768)
        vrows = qkv_w[2 * C + hs * HD: 2 * C + (hs + HPC) * HD]
        bq = qkv_b[hs * HD:(hs + HPC) * HD]
        bk = qkv_b[C + hs * HD: C + (hs + HPC) * HD]
        bvv = qkv_b[2 * C + hs * HD: 2 * C + (hs + HPC) * HD]

        packed = np.zeros((128, PACKED), f)
        packed[:, OFF_xT:OFF_xT + 6 * N] = _tiled_cols(np.asarray(x[b]).T.astype(f), 6)
        packed[:, OFF_wqkT:OFF_wqkT + 6 * 768] = _tiled_cols(
            np.concatenate([qrows, krows], 0).T.astype(f), 6)
        packed[:, OFF_wvT:OFF_wvT + 6 * 384] = _tiled_cols(vrows.T.astype(f), 6)
        packed[:, OFF_wpjT:OFF_wpjT + 3 * C] = _tiled_cols(
            np.asarray(proj_w).T[hs * HD:(hs + HPC) * HD, :].astype(f), 3)
        packed[:, OFF_ident:OFF_ident + 128] = np.eye(128, dtype=f)
        packed[:, OFF_bqk:OFF_bqk + 6] = np.concatenate([bq, bk]).reshape(6, 128).T
        packed[:, OFF_bvc:OFF_bvc + 3] = np.asarray(bvv).reshape(3, 128).T
        packed[0, OFF_ones:OFF_ones + 128] = 1.0
        packed[0, OFF_bv:OFF_bv + HPC * HD] = bvv
        packed[0, OFF_bpj:OFF_bpj + C] = np.asarray(proj_b) * 0.5
        packed[0, OFF_cls:OFF_cls + HPC] = cls_bias[hs:hs + HPC]
        in_maps.append({"inp": packed})
    return in_maps


_CACHED_NC = None


def _get_nc():
    global _CACHED_NC
    if _CACHED_NC is None:
        _CACHED_NC = build_bass()
    return _CACHED_NC


def run(trace=False, **inputs):
    nc = _get_nc()
    in_maps = make_in_maps(**inputs)
    res = bass_utils.run_bass_kernel_spmd(
        nc, in_maps, core_ids=list(range(NCORES)), trace=trace,
    )
    attn = np.empty((B, H, N, N), np.float32)
    out = np.empty((B, N, C), np.float32)
    for core in range(NCORES):
        b, g = core // 2, core % 2
        attn[b, g * HPC:(g + 1) * HPC] = res.results[core]["attn_out"]
        if g == 0:
            out[b] = res.results[core]["out_ext"]
    return (out, attn), res


def kernel(**inputs):
    outputs, _ = run(trace=False, **inputs)
    return outputs


# revision 39
# speedup vs baseline: 1.5584x; 1.0636x over previous
"""Trainium2 Bass kernel for a 12-head attention block with cls-token
rebalancing (B=4, N=1024, C=768), distributed over 8 NeuronCores.

Sharding: core = 2*b + g  (b = batch 0..3, g = head-group 0..1, 6 heads each).
Each core computes qkv / attention / softmax / cls-rebalance / attn@v for its
(batch, 6 heads), plus the partial output projection over its heads' channels;
core pairs {2b, 2b+1} AllReduce the projection partials.

Outputs: attn (4,12,1024,1024) fp32 and out (4,1024,768) fp32, matching
reference.py's (out, attn) tuple.
"""

import sys

if "/opt/trn_rl_repo" not in sys.path:
    sys.path.insert(0, "/opt/trn_rl_repo")

from contextlib import ExitStack

import numpy as np

import concourse.bass as bass
import concourse.bacc as bacc
import concourse.tile as tile
from concourse import mybir
from concourse import bass_utils

F32 = mybir.dt.float32
# TensorEngine compute dtype for the big matmuls: float32r is the fast PE path
# (1 cyc/row at free-dim >= 256 vs 4 for float32). Tensors feeding matmuls are
# tagged float32r natively so the BIR verifier sees rounded producers.
R32 = mybir.dt.float32r

B, N, C = 4, 1024, 768
H = 12
HPC = 6  # heads per core
HD = 64
SCALE = HD ** -0.5
EPS = 1e-6
NCORES = 8
REPLICA_GROUPS = [[0, 1], [2, 3], [4, 5], [6, 7]]

Exp = mybir.ActivationFunctionType.Exp
ALU = mybir.AluOpType

# packed-input column offsets (all fp32, 128 partitions)
OFF_xT = 0        # 6 c_in tiles x 1024 n
OFF_wqkT = 6144   # 6 c_in tiles x 768 qk cols
OFF_wvT = 10752   # 6 c_in tiles x 384 v cols
OFF_wpjT = 13056  # 3 c_in tiles x 768 cols
OFF_ident = 15360
OFF_bqk = 15488   # (128, 6)
OFF_bvc = 15494   # (128, 3)
OFF_ones = 15497  # row (partition 0)
OFF_bv = 15625    # row
OFF_bpj = 16009   # row
OFF_cls = 16777   # row
PACKED = 16784


def _mm(ap):
    return ap


def build_bass():
    nc = bacc.Bacc("TRN2", debug=False, target_bir_lowering=False, num_devices=NCORES)

    # ---- external I/O: ONE packed input tensor (single DMA -> single
    # semaphore lane, since the PE LDWEIGHTS slot only fits one sync wait) ----
    inp_d = nc.dram_tensor("inp", (128, PACKED), R32, kind="ExternalInput").ap()

    attn_d = nc.dram_tensor("attn_out", (HPC, N, N), F32, kind="ExternalOutput").ap()
    out_d = nc.dram_tensor("out_ext", (N, C), F32, kind="ExternalOutput").ap()
    dbg_d = None  # debug outputs disabled

    # ---- collective bounce buffers ----
    cc_in = nc.dram_tensor("cc_in", (N, C), F32).ap()
    cc_out = nc.dram_tensor("cc_out", (N, C), F32).ap()

    with tile.TileContext(nc) as tc, ExitStack() as ctx:
        P = ctx.enter_context  # pool opener

        persist = P(tc.tile_pool(name="persist", bufs=1))
        attn_pool = P(tc.tile_pool(name="attn", bufs=4))
        et_pool = P(tc.tile_pool(name="et", bufs=4))
        bc_pool = P(tc.tile_pool(name="bc", bufs=2))
        out_pool = P(tc.tile_pool(name="outsb", bufs=2))
        ps_pool = P(tc.tile_pool(name="ps", bufs=4, space="PSUM"))
        av_pool = ps_pool  # shared 4-slot rotation (8 psum banks total)

        # ---- persistent SBUF tensors ----
        inp = persist.tile([128, PACKED], R32, tag="inp")
        xT = inp[:, OFF_xT:OFF_xT + 6 * N]
        wqkT = inp[:, OFF_wqkT:OFF_wqkT + 6 * 768]
        wvT = inp[:, OFF_wvT:OFF_wvT + 6 * 384]
        wpjT = inp[:, OFF_wpjT:OFF_wpjT + 3 * C]
        ident = inp[:, OFF_ident:OFF_ident + 128].bitcast(F32)
        bqk = inp[:, OFF_bqk:OFF_bqk + 6].bitcast(F32)
        bvc = inp[:, OFF_bvc:OFF_bvc + 3].bitcast(F32)
        ones = inp[0:1, OFF_ones:OFF_ones + 128]
        bv = inp[0:1, OFF_bv:OFF_bv + HPC * HD]
        bpj = inp[0:1, OFF_bpj:OFF_bpj + C]
        clsb = inp[0:1, OFF_cls:OFF_cls + HPC].bitcast(F32)

        qkvT = persist.tile([128, 6 * N], R32, tag="qkvT")       # m 0..2: q pairs, 3..5: k pairs
        vsb = persist.tile([128, 8 * 384], R32, tag="vsb")       # 8 n tiles x (6 heads*64)
        outT = persist.tile([128, 3 * N], R32, tag="outT")       # pair j: c_in x n
        S = persist.tile([128, HPC * 8], F32, tag="S")           # row sums, head h cols h*8..
        iS = persist.tile([128, HPC * 8], F32, tag="iS")         # 1/S
        iSr = persist.tile([1, HPC * N], F32, tag="iSr")         # transposed 1/S rows
        v0T = persist.tile([128, 3], F32, tag="v0T")             # v[0,:] as columns
        cells = persist.tile([1, 8 * HPC], F32, tag="cells")     # per-head scalars

        # ---- one DMA for all inputs ----
        nc.sync.dma_start(out=inp[:, :], in_=inp_d[:, :])

        # ---- qk projection: qkvT[m] = (wqk tile m).T @ x, in [c_out, n] layout ----
        for m in range(6):
            ps = ps_pool.tile([128, N], F32, tag="ps")
            for nh in range(2):
                for k in range(6):
                    nc.tensor.matmul(
                        ps[:, nh * 512:(nh + 1) * 512],
                        _mm(wqkT[:, k * 768 + m * 128: k * 768 + (m + 1) * 128]),
                        _mm(xT[:, k * N + nh * 512: k * N + (nh + 1) * 512]),
                        start=(k == 0), stop=(k == 5),
                    )
            nc.vector.tensor_scalar_add(qkvT[:, m * N:(m + 1) * N], ps[:, :], bqk[:, m:m + 1])

        # ---- v projection in natural [n, d] layout: v = x.T.T @ wvT ----
        for nt in range(8):
            ps = ps_pool.tile([128, N], F32, tag="ps")
            for k in range(6):
                nc.tensor.matmul(
                    ps[:, 0:384],
                    _mm(xT[:, k * N + nt * 128: k * N + (nt + 1) * 128]),
                    _mm(wvT[:, k * 384:(k + 1) * 384]),
                    start=(k == 0), stop=False,
                )
            # + bias row via rank-1 ones matmul
            nc.tensor.matmul(ps[:, 0:384], _mm(ones[0:1, :]), _mm(bv[0:1, :]),
                             start=False, stop=True)
            nc.vector.tensor_copy(vsb[:, nt * 384:(nt + 1) * 384], ps[:, 0:384])

        # ---- v0T: v[0, :] as [d, 1] columns (for the cls row-0 output fix) ----
        for mt in range(3):
            ps = ps_pool.tile([128, N], F32, tag="ps")
            for k in range(6):
                nc.tensor.matmul(
                    ps[:, 0:1],
                    wvT[:, k * 384 + mt * 128: k * 384 + (mt + 1) * 128].bitcast(F32),
                    xT[:, k * N: k * N + 1].bitcast(F32),
                    start=(k == 0), stop=(k == 5),
                )
            nc.vector.tensor_scalar_add(v0T[:, mt:mt + 1], ps[:, 0:1], bvc[:, mt:mt + 1])

        # ---- attention per head-pair ----
        for j in range(3):
            qt_pair = qkvT[:, j * N:(j + 1) * N]
            kt_pair = qkvT[:, (3 + j) * N:(4 + j) * N]

            # pass 1: scores in [q, k] layout, softmax, cls-rebalance, attn out
            for half in range(2):
                h = 2 * j + half
                rows = slice(64 * half, 64 * half + 64)
                for qt in range(8):
                    ps = ps_pool.tile([128, N], F32, tag="ps")
                    for kh in range(2):
                        nc.tensor.matmul(
                            ps[:, kh * 512:(kh + 1) * 512],
                            _mm(qt_pair[rows, qt * 128:(qt + 1) * 128]),
                            _mm(kt_pair[rows, kh * 512:(kh + 1) * 512]),
                            start=True, stop=True,
                        )
                    at = attn_pool.tile([128, N], F32, tag="attn")
                    sc = S[:, h * 8 + qt: h * 8 + qt + 1]
                    nc.scalar.activation(at[:, :], ps[:, :], Exp, scale=SCALE, accum_out=sc)
                    isc = iS[:, h * 8 + qt: h * 8 + qt + 1]
                    nc.vector.reciprocal(isc, sc)
                    nc.vector.tensor_scalar_mul(at[:, :], at[:, :], isc)

                    if qt == 0:
                        # cls-token rebalancing on row 0 (partition 0 of tile 0)
                        cb = cells[0:1, h * 8: h * 8 + 8]
                        a00 = at[0:1, 0:1]
                        # c0 = a00_new = min(attn00 + cls_bias, 1)
                        nc.vector.tensor_scalar(cb[0:1, 0:1], a00, clsb[0:1, h:h + 1], 1.0,
                                                op0=ALU.add, op1=ALU.min)
                        # c1 = denom = (1 + eps) - attn00   (= actual + eps)
                        nc.vector.tensor_scalar(cb[0:1, 1:2], a00, -1.0, 1.0 + EPS,
                                                op0=ALU.mult, op1=ALU.add)
                        # c2 = 1/denom
                        nc.vector.reciprocal(cb[0:1, 2:3], cb[0:1, 1:2])
                        # c3 = 1 - a00_new
                        nc.vector.tensor_scalar(cb[0:1, 3:4], cb[0:1, 0:1], -1.0, 1.0,
                                                op0=ALU.mult, op1=ALU.add)
                        # c4 = mp = (1 - a00_new) / denom
                        nc.vector.tensor_mul(cb[0:1, 4:5], cb[0:1, 3:4], cb[0:1, 2:3])
                        # c5 = mp * attn00
                        nc.vector.tensor_mul(cb[0:1, 5:6], cb[0:1, 4:5], a00)
                        # c6 = beta = a00_new - mp * attn00
                        nc.vector.tensor_sub(cb[0:1, 6:7], cb[0:1, 0:1], cb[0:1, 5:6])
                        # rescale rest of row 0 by mp, then set attn[0,0] = a00_new
                        nc.vector.tensor_scalar_mul(at[0:1, 1:N], at[0:1, 1:N], cb[0:1, 4:5])
                        nc.vector.tensor_copy(at[0:1, 0:1], cb[0:1, 0:1])

                    nc.sync.dma_start(out=attn_d[h, qt * 128:(qt + 1) * 128, :], in_=at[:, :])

                # transpose invS for this head: 8x ([128,1] -> [1,128]) on row 0
                ps = ps_pool.tile([128, N], F32, tag="ps")
                for qt in range(8):
                    nc.tensor.transpose(ps[0:1, qt * 128:(qt + 1) * 128],
                                        iS[:, h * 8 + qt: h * 8 + qt + 1], ident[:, :])
                nc.vector.tensor_copy(iSr[0:1, h * N:(h + 1) * N], ps[0:1, :])

            # pass 2: scores.T in [k, q] layout, exp, attn@v (PE col-packed pair).
            # A and B halves accumulate in SEPARATE psum tiles: hardware allows
            # only one matmul accumulation group per psum bank at a time.
            avt = [av_pool.tile([128, N], F32, name=f"avt{_h}", tag="ps") for _h in range(2)]
            for kt in range(8):
                for half in range(2):
                    h = 2 * j + half
                    rows = slice(64 * half, 64 * half + 64)
                    ps = ps_pool.tile([128, N], F32, tag="ps")
                    for qh in range(2):
                        nc.tensor.matmul(
                            ps[:, qh * 512:(qh + 1) * 512],
                            _mm(kt_pair[rows, kt * 128:(kt + 1) * 128]),
                            _mm(qt_pair[rows, qh * 512:(qh + 1) * 512]),
                            start=True, stop=True,
                        )
                    et = et_pool.tile([128, N], R32, tag="et")
                    nc.scalar.activation(et[:, :], ps[:, :], Exp, scale=SCALE)
                    # stationary operand = both heads' v columns (128 wide);
                    # only this half's 64 output rows are meaningful, the other
                    # 64 rows of avt[half] are never read.
                    vcol = vsb[:, kt * 384 + j * 128: kt * 384 + (j + 1) * 128]
                    for qh in range(2):
                        nc.tensor.matmul(
                            avt[half][:, qh * 512:(qh + 1) * 512],
                            _mm(vcol),
                            _mm(et[:, qh * 512:(qh + 1) * 512]),
                            start=(kt == 0), stop=(kt == 7),
                        )

            # normalize attn@v by 1/S[q] (free-dim scalar via partition bcast;
            # partition_broadcast writes from partition 0, so broadcast each
            # head's row to ALL partitions and slice the half we need)
            for c in range(8):
                for half in range(2):
                    h = 2 * j + half
                    rows = slice(64 * half, 64 * half + 64)
                    bc = bc_pool.tile([128, 128], F32, name=f"bc{half}", tag="bc")
                    nc.gpsimd.partition_broadcast(
                        bc[:, :], iSr[0:1, h * N + c * 128: h * N + (c + 1) * 128])
                    nc.vector.tensor_mul(
                        outT[rows, j * N + c * 128: j * N + (c + 1) * 128],
                        avt[half][rows, c * 128:(c + 1) * 128], bc[rows, :])

            # cls row-0 output fix: out[0,:] = mp*out[0,:] + beta*v[0,:]
            for half in range(2):
                h = 2 * j + half
                rows = slice(64 * half, 64 * half + 64)
                bc = bc_pool.tile([128, 128], F32, name=f"bcf{half}", tag="bc")
                nc.gpsimd.partition_broadcast(bc[:, 0:1], cells[0:1, h * 8 + 4: h * 8 + 5])
                nc.gpsimd.partition_broadcast(bc[:, 1:2], cells[0:1, h * 8 + 6: h * 8 + 7])
                v0 = v0T[rows, j: j + 1]
                col0 = outT[rows, j * N: j * N + 1]
                # bc[:,2] = beta * v0
                nc.vector.tensor_scalar_mul(bc[rows, 2:3], v0, bc[rows, 1:2])
                # col0 = mp * col0 + beta*v0
                nc.vector.scalar_tensor_tensor(col0, col0, bc[rows, 0:1], bc[rows, 2:3],
                                               op0=ALU.mult, op1=ALU.add)

        # ---- output projection partial: out_part[n, c] over this group's c_in ----
        for nt in range(8):
            ps = ps_pool.tile([128, N], F32, tag="ps")
            for ch in range(2):
                # bank-aligned regions: [0:384] in bank 0, [512:896] in bank 1
                cs = slice(ch * 512, ch * 512 + 384)
                for ktj in range(3):
                    nc.tensor.matmul(
                        ps[:, cs],
                        _mm(outT[:, ktj * N + nt * 128: ktj * N + (nt + 1) * 128]),
                        _mm(wpjT[:, ktj * C + ch * 384: ktj * C + (ch + 1) * 384]),
                        start=(ktj == 0), stop=False,
                    )
                nc.tensor.matmul(ps[:, cs], _mm(ones[0:1, :]),
                                 _mm(bpj[0:1, ch * 384:(ch + 1) * 384]),
                                 start=False, stop=True)
            ot = out_pool.tile([128, C], F32, tag="outsb")
            nc.vector.tensor_copy(ot[:, 0:384], ps[:, 0:384])
            nc.vector.tensor_copy(ot[:, 384:768], ps[:, 512:896])
            nc.sync.dma_start(out=cc_in[nt * 128:(nt + 1) * 128, :], in_=ot[:, :])

        # ---- pair AllReduce of projection partials (4 chunks, overlapped) ----
        for cchunk in range(4):
            rs = slice(cchunk * 256, (cchunk + 1) * 256)
            nc.gpsimd.collective_compute(
                "AllReduce", ALU.add, replica_groups=REPLICA_GROUPS,
                ins=[cc_in[rs, :].opt()], outs=[cc_out[rs, :].opt()],
            )
            nc.sync.dma_start(out=out_d[rs, :], in_=cc_out[rs, :])


    nc.compile()
    _split_waits(nc)
    return nc


def _tiled_cols(a, kk):
    """(kk*128, M) -> (128, kk*M): column block k = rows k*128..(k+1)*128."""
    m = a.shape[1]
    return a.reshape(kk, 128, m).transpose(1, 0, 2).reshape(128, kk * m)


def _split_waits(nc):
    """Walrus codegen caps sync-waits at 1 per instruction (2 for
    EventSemaphore). Spill extra waits onto EventSemaphore NOPs inserted
    just before, on the same engine stream."""
    nid = [0]

    def nop_with(engine, waits):
        nid[0] += 1
        nop = mybir.InstEventSemaphore(name=f"WSPILL-{nid[0]}", ins=[], outs=[])
        nop.engine = engine
        nop.sync_info = mybir.SyncInfo(on_wait=list(waits), on_update=[])
        return nop

    for f in nc.m.functions:
        for blk in f.blocks:
            out = []
            changed = False
            for inst in blk.instructions:
                si = inst.sync_info
                waits = list(si.on_wait) if si is not None and si.on_wait else []
                cap = 2 if isinstance(inst, mybir.InstEventSemaphore) else 1
                if len(waits) > cap:
                    spill, keep = waits[:-cap], waits[-cap:]
                    for i in range(0, len(spill), 2):
                        out.append(nop_with(inst.engine, spill[i:i + 2]))
                    inst.sync_info = mybir.SyncInfo(
                        on_wait=keep, on_update=list(si.on_update) if si.on_update else [])
                    changed = True
                out.append(inst)
            if changed:
                blk.instructions = out


def make_in_maps(x, qkv_w, qkv_b, proj_w, proj_b, cls_bias):
    f = np.float32
    in_maps = []
    for core in range(NCORES):
        b, g = core // 2, core % 2
        hs = g * HPC
        qrows = qkv_w[hs * HD:(hs + HPC) * HD]            # (384, 768)
        krows = qkv_w[C + hs * HD: C + (hs + HPC) * HD]   # (384, 768)
        vrows = qkv_w[2 * C + hs * HD: 2 * C + (hs + HPC) * HD]
        bq = qkv_b[hs * HD:(hs + HPC) * HD]
        bk = qkv_b[C + hs * HD: C + (hs + HPC) * HD]
        bvv = qkv_b[2 * C + hs * HD: 2 * C + (hs + HPC) * HD]

        packed = np.zeros((128, PACKED), f)
        packed[:, OFF_xT:OFF_xT + 6 * N] = _tiled_cols(np.asarray(x[b]).T.astype(f), 6)
        packed[:, OFF_wqkT:OFF_wqkT + 6 * 768] = _tiled_cols(
            np.concatenate([qrows, krows], 0).T.astype(f), 6)
        packed[:, OFF_wvT:OFF_wvT + 6 * 384] = _tiled_cols(vrows.T.astype(f), 6)
        packed[:, OFF_wpjT:OFF_wpjT + 3 * C] = _tiled_cols(
            np.asarray(proj_w).T[hs * HD:(hs + HPC) * HD, :].astype(f), 3)
        packed[:, OFF_ident:OFF_ident + 128] = np.eye(128, dtype=f)
        packed[:, OFF_bqk:OFF_bqk + 6] = np.concatenate([bq, bk]).reshape(6, 128).T
        packed[:, OFF_bvc:OFF_bvc + 3] = np.asarray(bvv).reshape(3, 128).T
        packed[0, OFF_ones:OFF_ones + 128] = 1.0
        packed[0, OFF_bv:OFF_bv + HPC * HD] = bvv
        packed[0, OFF_bpj:OFF_bpj + C] = np.asarray(proj_b) * 0.5
        packed[0, OFF_cls:OFF_cls + HPC] = cls_bias[hs:hs + HPC]
        in_maps.append({"inp": packed})
    return in_maps


_CACHED_NC = None


def _get_nc():
    global _CACHED_NC
    if _CACHED_NC is None:
        _CACHED_NC = build_bass()
    return _CACHED_NC


def run(trace=False, **inputs):
    nc = _get_nc()
    in_maps = make_in_maps(**inputs)
    res = bass_utils.run_bass_kernel_spmd(
        nc, in_maps, core_ids=list(range(NCORES)), trace=trace,
    )
    attn = np.empty((B, H, N, N), np.float32)
    out = np.empty((B, N, C), np.float32)
    for core in range(NCORES):
        b, g = core // 2, core % 2
        attn[b, g * HPC:(g + 1) * HPC] = res.results[core]["attn_out"]
        if g == 0:
            out[b] = res.results[core]["out_ext"]
    return (out, attn), res


def kernel(**inputs):
    outputs, _ = run(trace=False, **inputs)
    return outputs


# revision 41
# speedup vs baseline: 1.6964x; 1.0886x over previous
"""Trainium2 Bass kernel for a 12-head attention block with cls-token
rebalancing (B=4, N=1024, C=768), distributed over 8 NeuronCores.

Sharding: core = 2*b + g  (b = batch 0..3, g = head-group 0..1, 6 heads each).
Each core computes qkv / attention / softmax / cls-rebalance / attn@v for its
(batch, 6 heads), plus the partial output projection over its heads' channels;
core pairs {2b, 2b+1} AllReduce the projection partials.

Outputs: attn (4,12,1024,1024) fp32 and out (4,1024,768) fp32, matching
reference.py's (out, attn) tuple.
"""

import sys

if "/opt/trn_rl_repo" not in sys.path:
    sys.path.insert(0, "/opt/trn_rl_repo")

from contextlib import ExitStack

import numpy as np

import concourse.bass as bass
import concourse.bacc as bacc
import concourse.tile as tile
from concourse import mybir
from concourse import bass_utils

F32 = mybir.dt.float32
# Matmul payload runs in bf16: fast weight load (FWL), 1 cyc/row, half the DMA.
BF16 = mybir.dt.bfloat16

B, N, C = 4, 1024, 768
H = 12
HPC = 6  # heads per core
HD = 64
SCALE = HD ** -0.5
EPS = 1e-6
NCORES = 8
REPLICA_GROUPS = [[0, 1], [2, 3], [4, 5], [6, 7]]

Exp = mybir.ActivationFunctionType.Exp
ALU = mybir.AluOpType

# bf16 packed-input column offsets (128 partitions)
OFF_xT = 0        # 6 c_in tiles x 1024 n
OFF_wqkT = 6144   # 6 c_in tiles x 768 qk cols
OFF_wvT = 10752   # 6 c_in tiles x 384 v cols
OFF_wpjT = 13056  # 3 c_in tiles x 768 cols
OFF_ones = 15360  # row (partition 0)
OFF_bv = 15488    # row
OFF_bpj = 15872   # row
PACKED = 16640
# fp32 small-constant input offsets
SOFF_ident = 0
SOFF_bqk = 128    # (128, 6)
SOFF_bvc = 134    # (128, 3)
SOFF_cls = 137    # row
SPACKED = 144


def _mm(ap):
    return ap


def build_bass():
    nc = bacc.Bacc("TRN2", debug=False, target_bir_lowering=False, num_devices=NCORES)

    # ---- external I/O: ONE packed input tensor (single DMA -> single
    # semaphore lane, since the PE LDWEIGHTS slot only fits one sync wait) ----
    inp_d = nc.dram_tensor("inp", (128, PACKED), BF16, kind="ExternalInput").ap()
    inps_d = nc.dram_tensor("inps", (128, SPACKED), F32, kind="ExternalInput").ap()

    attn_d = nc.dram_tensor("attn_out", (HPC, N, N), F32, kind="ExternalOutput").ap()
    out_d = nc.dram_tensor("out_ext", (N, C), F32, kind="ExternalOutput").ap()
    dbg_d = None  # debug outputs disabled

    # ---- collective bounce buffers ----
    cc_in = nc.dram_tensor("cc_in", (N, C), F32).ap()
    cc_out = nc.dram_tensor("cc_out", (N, C), F32).ap()

    with tile.TileContext(nc) as tc, ExitStack() as ctx:
        P = ctx.enter_context  # pool opener

        persist = P(tc.tile_pool(name="persist", bufs=1))
        attn_pool = P(tc.tile_pool(name="attn", bufs=4))
        et_pool = P(tc.tile_pool(name="et", bufs=4))
        bc_pool = P(tc.tile_pool(name="bc", bufs=2))
        out_pool = P(tc.tile_pool(name="outsb", bufs=2))
        ps_pool = P(tc.tile_pool(name="ps", bufs=4, space="PSUM"))
        av_pool = ps_pool  # shared 4-slot rotation (8 psum banks total)

        # ---- persistent SBUF tensors ----
        inp = persist.tile([128, PACKED], BF16, tag="inp")
        inps = persist.tile([128, SPACKED], F32, tag="inps")
        xT = inp[:, OFF_xT:OFF_xT + 6 * N]
        wqkT = inp[:, OFF_wqkT:OFF_wqkT + 6 * 768]
        wvT = inp[:, OFF_wvT:OFF_wvT + 6 * 384]
        wpjT = inp[:, OFF_wpjT:OFF_wpjT + 3 * C]
        ones = inp[0:1, OFF_ones:OFF_ones + 128]
        bv = inp[0:1, OFF_bv:OFF_bv + HPC * HD]
        bpj = inp[0:1, OFF_bpj:OFF_bpj + C]
        ident = inps[:, SOFF_ident:SOFF_ident + 128]
        bqk = inps[:, SOFF_bqk:SOFF_bqk + 6]
        bvc = inps[:, SOFF_bvc:SOFF_bvc + 3]
        clsb = inps[0:1, SOFF_cls:SOFF_cls + HPC]

        qkvT = persist.tile([128, 6 * N], BF16, tag="qkvT")       # m 0..2: q pairs, 3..5: k pairs
        vsb = persist.tile([128, 8 * 384], BF16, tag="vsb")       # 8 n tiles x (6 heads*64)
        outT = persist.tile([128, 3 * N], BF16, tag="outT")       # pair j: c_in x n
        S = persist.tile([128, HPC * 8], F32, tag="S")           # row sums, head h cols h*8..
        iS = persist.tile([128, HPC * 8], F32, tag="iS")         # 1/S
        iSr = persist.tile([1, HPC * N], F32, tag="iSr")         # transposed 1/S rows
        v0T = persist.tile([128, 3], F32, tag="v0T")             # v[0,:] as columns
        cells = persist.tile([1, 8 * HPC], F32, tag="cells")     # per-head scalars

        # ---- input DMAs ----
        nc.sync.dma_start(out=inp[:, :], in_=inp_d[:, :])
        nc.sync.dma_start(out=inps[:, :], in_=inps_d[:, :])

        # ---- qk projection: qkvT[m] = (wqk tile m).T @ x, in [c_out, n] layout ----
        for m in range(6):
            ps = ps_pool.tile([128, N], F32, tag="ps")
            for nh in range(2):
                for k in range(6):
                    nc.tensor.matmul(
                        ps[:, nh * 512:(nh + 1) * 512],
                        _mm(wqkT[:, k * 768 + m * 128: k * 768 + (m + 1) * 128]),
                        _mm(xT[:, k * N + nh * 512: k * N + (nh + 1) * 512]),
                        start=(k == 0), stop=(k == 5),
                    )
            nc.vector.tensor_scalar_add(qkvT[:, m * N:(m + 1) * N], ps[:, :], bqk[:, m:m + 1])

        # ---- v projection in natural [n, d] layout: v = x.T.T @ wvT ----
        for nt in range(8):
            ps = ps_pool.tile([128, N], F32, tag="ps")
            for k in range(6):
                nc.tensor.matmul(
                    ps[:, 0:384],
                    _mm(xT[:, k * N + nt * 128: k * N + (nt + 1) * 128]),
                    _mm(wvT[:, k * 384:(k + 1) * 384]),
                    start=(k == 0), stop=False,
                )
            # + bias row via rank-1 ones matmul
            nc.tensor.matmul(ps[:, 0:384], _mm(ones[0:1, :]), _mm(bv[0:1, :]),
                             start=False, stop=True)
            nc.vector.tensor_copy(vsb[:, nt * 384:(nt + 1) * 384], ps[:, 0:384])

        # ---- v0T: v[0, :] as [d, 1] columns (for the cls row-0 output fix) ----
        for mt in range(3):
            ps = ps_pool.tile([128, N], F32, tag="ps")
            for k in range(6):
                nc.tensor.matmul(
                    ps[:, 0:1],
                    wvT[:, k * 384 + mt * 128: k * 384 + (mt + 1) * 128],
                    xT[:, k * N: k * N + 1],
                    start=(k == 0), stop=(k == 5),
                )
            nc.vector.tensor_scalar_add(v0T[:, mt:mt + 1], ps[:, 0:1], bvc[:, mt:mt + 1])

        # ---- attention per head-pair ----
        for j in range(3):
            qt_pair = qkvT[:, j * N:(j + 1) * N]
            kt_pair = qkvT[:, (3 + j) * N:(4 + j) * N]

            # pass 1: scores in [q, k] layout, softmax, cls-rebalance, attn out
            for half in range(2):
                h = 2 * j + half
                rows = slice(64 * half, 64 * half + 64)
                for qt in range(8):
                    ps = ps_pool.tile([128, N], F32, tag="ps")
                    for kh in range(2):
                        nc.tensor.matmul(
                            ps[:, kh * 512:(kh + 1) * 512],
                            _mm(qt_pair[rows, qt * 128:(qt + 1) * 128]),
                            _mm(kt_pair[rows, kh * 512:(kh + 1) * 512]),
                            start=True, stop=True,
                        )
                    at = attn_pool.tile([128, N], F32, tag="attn")
                    sc = S[:, h * 8 + qt: h * 8 + qt + 1]
                    nc.scalar.activation(at[:, :], ps[:, :], Exp, scale=SCALE, accum_out=sc)
                    isc = iS[:, h * 8 + qt: h * 8 + qt + 1]
                    nc.vector.reciprocal(isc, sc)
                    nc.vector.tensor_scalar_mul(at[:, :], at[:, :], isc)

                    if qt == 0:
                        # cls-token rebalancing on row 0 (partition 0 of tile 0)
                        cb = cells[0:1, h * 8: h * 8 + 8]
                        a00 = at[0:1, 0:1]
                        # c0 = a00_new = min(attn00 + cls_bias, 1)
                        nc.vector.tensor_scalar(cb[0:1, 0:1], a00, clsb[0:1, h:h + 1], 1.0,
                                                op0=ALU.add, op1=ALU.min)
                        # c1 = denom = (1 + eps) - attn00   (= actual + eps)
                        nc.vector.tensor_scalar(cb[0:1, 1:2], a00, -1.0, 1.0 + EPS,
                                                op0=ALU.mult, op1=ALU.add)
                        # c2 = 1/denom
                        nc.vector.reciprocal(cb[0:1, 2:3], cb[0:1, 1:2])
                        # c3 = 1 - a00_new
                        nc.vector.tensor_scalar(cb[0:1, 3:4], cb[0:1, 0:1], -1.0, 1.0,
                                                op0=ALU.mult, op1=ALU.add)
                        # c4 = mp = (1 - a00_new) / denom
                        nc.vector.tensor_mul(cb[0:1, 4:5], cb[0:1, 3:4], cb[0:1, 2:3])
                        # c5 = mp * attn00
                        nc.vector.tensor_mul(cb[0:1, 5:6], cb[0:1, 4:5], a00)
                        # c6 = beta = a00_new - mp * attn00
                        nc.vector.tensor_sub(cb[0:1, 6:7], cb[0:1, 0:1], cb[0:1, 5:6])
                        # rescale rest of row 0 by mp, then set attn[0,0] = a00_new
                        nc.vector.tensor_scalar_mul(at[0:1, 1:N], at[0:1, 1:N], cb[0:1, 4:5])
                        nc.vector.tensor_copy(at[0:1, 0:1], cb[0:1, 0:1])

                    nc.sync.dma_start(out=attn_d[h, qt * 128:(qt + 1) * 128, :], in_=at[:, :])

                # transpose invS for this head: 8x ([128,1] -> [1,128]) on row 0
                ps = ps_pool.tile([128, N], F32, tag="ps")
                for qt in range(8):
                    nc.tensor.transpose(ps[0:1, qt * 128:(qt + 1) * 128],
                                        iS[:, h * 8 + qt: h * 8 + qt + 1], ident[:, :])
                nc.vector.tensor_copy(iSr[0:1, h * N:(h + 1) * N], ps[0:1, :])

            # pass 2: scores.T in [k, q] layout, exp, attn@v (PE col-packed pair).
            # A and B halves accumulate in SEPARATE psum tiles: hardware allows
            # only one matmul accumulation group per psum bank at a time.
            avt = [av_pool.tile([128, N], F32, name=f"avt{_h}", tag="ps") for _h in range(2)]
            for kt in range(8):
                for half in range(2):
                    h = 2 * j + half
                    rows = slice(64 * half, 64 * half + 64)
                    ps = ps_pool.tile([128, N], F32, tag="ps")
                    for qh in range(2):
                        nc.tensor.matmul(
                            ps[:, qh * 512:(qh + 1) * 512],
                            _mm(kt_pair[rows, kt * 128:(kt + 1) * 128]),
                            _mm(qt_pair[rows, qh * 512:(qh + 1) * 512]),
                            start=True, stop=True,
                        )
                    et = et_pool.tile([128, N], BF16, tag="et")
                    nc.scalar.activation(et[:, :], ps[:, :], Exp, scale=SCALE)
                    # stationary operand = both heads' v columns (128 wide);
                    # only this half's 64 output rows are meaningful, the other
                    # 64 rows of avt[half] are never read.
                    vcol = vsb[:, kt * 384 + j * 128: kt * 384 + (j + 1) * 128]
                    for qh in range(2):
                        nc.tensor.matmul(
                            avt[half][:, qh * 512:(qh + 1) * 512],
                            _mm(vcol),
                            _mm(et[:, qh * 512:(qh + 1) * 512]),
                            start=(kt == 0), stop=(kt == 7),
                        )

            # normalize attn@v by 1/S[q] (free-dim scalar via partition bcast;
            # partition_broadcast writes from partition 0, so broadcast each
            # head's row to ALL partitions and slice the half we need)
            for c in range(8):
                for half in range(2):
                    h = 2 * j + half
                    rows = slice(64 * half, 64 * half + 64)
                    bc = bc_pool.tile([128, 128], F32, name=f"bc{half}", tag="bc")
                    nc.gpsimd.partition_broadcast(
                        bc[:, :], iSr[0:1, h * N + c * 128: h * N + (c + 1) * 128])
                    nc.vector.tensor_mul(
                        outT[rows, j * N + c * 128: j * N + (c + 1) * 128],
                        avt[half][rows, c * 128:(c + 1) * 128], bc[rows, :])

            # cls row-0 output fix: out[0,:] = mp*out[0,:] + beta*v[0,:]
            for half in range(2):
                h = 2 * j + half
                rows = slice(64 * half, 64 * half + 64)
                bc = bc_pool.tile([128, 128], F32, name=f"bcf{half}", tag="bc")
                nc.gpsimd.partition_broadcast(bc[:, 0:1], cells[0:1, h * 8 + 4: h * 8 + 5])
                nc.gpsimd.partition_broadcast(bc[:, 1:2], cells[0:1, h * 8 + 6: h * 8 + 7])
                v0 = v0T[rows, j: j + 1]
                col0 = outT[rows, j * N: j * N + 1]
                # bc[:,2] = beta * v0
                nc.vector.tensor_scalar_mul(bc[rows, 2:3], v0, bc[rows, 1:2])
                # col0 = mp * col0 + beta*v0
                nc.vector.scalar_tensor_tensor(col0, col0, bc[rows, 0:1], bc[rows, 2:3],
                                               op0=ALU.mult, op1=ALU.add)

        # ---- output projection partial: out_part[n, c] over this group's c_in ----
        for nt in range(8):
            ps = ps_pool.tile([128, N], F32, tag="ps")
            for ch in range(2):
                # bank-aligned regions: [0:384] in bank 0, [512:896] in bank 1
                cs = slice(ch * 512, ch * 512 + 384)
                for ktj in range(3):
                    nc.tensor.matmul(
                        ps[:, cs],
                        _mm(outT[:, ktj * N + nt * 128: ktj * N + (nt + 1) * 128]),
                        _mm(wpjT[:, ktj * C + ch * 384: ktj * C + (ch + 1) * 384]),
                        start=(ktj == 0), stop=False,
                    )
                nc.tensor.matmul(ps[:, cs], _mm(ones[0:1, :]),
                                 _mm(bpj[0:1, ch * 384:(ch + 1) * 384]),
                                 start=False, stop=True)
            ot = out_pool.tile([128, C], F32, tag="outsb")
            nc.vector.tensor_copy(ot[:, 0:384], ps[:, 0:384])
            nc.vector.tensor_copy(ot[:, 384:768], ps[:, 512:896])
            nc.sync.dma_start(out=cc_in[nt * 128:(nt + 1) * 128, :], in_=ot[:, :])

        # ---- pair AllReduce of projection partials (4 chunks, overlapped) ----
        for cchunk in range(4):
            rs = slice(cchunk * 256, (cchunk + 1) * 256)
            nc.gpsimd.collective_compute(
                "AllReduce", ALU.add, replica_groups=REPLICA_GROUPS,
                ins=[cc_in[rs, :].opt()], outs=[cc_out[rs, :].opt()],
            )
            nc.sync.dma_start(out=out_d[rs, :], in_=cc_out[rs, :])


    nc.compile()
    _split_waits(nc)
    return nc


def _tiled_cols(a, kk):
    """(kk*128, M) -> (128, kk*M): column block k = rows k*128..(k+1)*128."""
    m = a.shape[1]
    return a.reshape(kk, 128, m).transpose(1, 0, 2).reshape(128, kk * m)


def _split_waits(nc):
    """Walrus codegen caps sync-waits at 1 per instruction (2 for
    EventSemaphore). Spill extra waits onto EventSemaphore NOPs inserted
    just before, on the same engine stream."""
    nid = [0]

    def nop_with(engine, waits):
        nid[0] += 1
        nop = mybir.InstEventSemaphore(name=f"WSPILL-{nid[0]}", ins=[], outs=[])
        nop.engine = engine
        nop.sync_info = mybir.SyncInfo(on_wait=list(waits), on_update=[])
        return nop

    for f in nc.m.functions:
        for blk in f.blocks:
            out = []
            changed = False
            for inst in blk.instructions:
                si = inst.sync_info
                waits = list(si.on_wait) if si is not None and si.on_wait else []
                cap = 2 if isinstance(inst, mybir.InstEventSemaphore) else 1
                if len(waits) > cap:
                    spill, keep = waits[:-cap], waits[-cap:]
                    for i in range(0, len(spill), 2):
                        out.append(nop_with(inst.engine, spill[i:i + 2]))
                    inst.sync_info = mybir.SyncInfo(
                        on_wait=keep, on_update=list(si.on_update) if si.on_update else [])
                    changed = True
                out.append(inst)
            if changed:
                blk.instructions = out


def make_in_maps(x, qkv_w, qkv_b, proj_w, proj_b, cls_bias):
    import ml_dtypes
    f = np.float32
    bf = ml_dtypes.bfloat16
    in_maps = []
    for core in range(NCORES):
        b, g = core // 2, core % 2
        hs = g * HPC
        qrows = qkv_w[hs * HD:(hs + HPC) * HD]            # (384, 768)
        krows = qkv_w[C + hs * HD: C + (hs + HPC) * HD]   # (384, 768)
        vrows = qkv_w[2 * C + hs * HD: 2 * C + (hs + HPC) * HD]
        bq = qkv_b[hs * HD:(hs + HPC) * HD]
        bk = qkv_b[C + hs * HD: C + (hs + HPC) * HD]
        bvv = qkv_b[2 * C + hs * HD: 2 * C + (hs + HPC) * HD]

        packed = np.zeros((128, PACKED), f)
        packed[:, OFF_xT:OFF_xT + 6 * N] = _tiled_cols(np.asarray(x[b]).T.astype(f), 6)
        packed[:, OFF_wqkT:OFF_wqkT + 6 * 768] = _tiled_cols(
            np.concatenate([qrows, krows], 0).T.astype(f), 6)
        packed[:, OFF_wvT:OFF_wvT + 6 * 384] = _tiled_cols(vrows.T.astype(f), 6)
        packed[:, OFF_wpjT:OFF_wpjT + 3 * C] = _tiled_cols(
            np.asarray(proj_w).T[hs * HD:(hs + HPC) * HD, :].astype(f), 3)
        packed[0, OFF_ones:OFF_ones + 128] = 1.0
        packed[0, OFF_bv:OFF_bv + HPC * HD] = bvv
        packed[0, OFF_bpj:OFF_bpj + C] = np.asarray(proj_b) * 0.5

        small = np.zeros((128, SPACKED), f)
        small[:, SOFF_ident:SOFF_ident + 128] = np.eye(128, dtype=f)
        small[:, SOFF_bqk:SOFF_bqk + 6] = np.concatenate([bq, bk]).reshape(6, 128).T
        small[:, SOFF_bvc:SOFF_bvc + 3] = np.asarray(bvv).reshape(3, 128).T
        small[0, SOFF_cls:SOFF_cls + HPC] = cls_bias[hs:hs + HPC]
        in_maps.append({"inp": packed.astype(bf), "inps": small})
    return in_maps


_CACHED_NC = None


def _get_nc():
    global _CACHED_NC
    if _CACHED_NC is None:
        _CACHED_NC = build_bass()
    return _CACHED_NC


def run(trace=False, **inputs):
    nc = _get_nc()
    in_maps = make_in_maps(**inputs)
    res = bass_utils.run_bass_kernel_spmd(
        nc, in_maps, core_ids=list(range(NCORES)), trace=trace,
    )
    attn = np.empty((B, H, N, N), np.float32)
    out = np.empty((B, N, C), np.float32)
    for core in range(NCORES):
        b, g = core // 2, core % 2
        attn[b, g * HPC:(g + 1) * HPC] = res.results[core]["attn_out"]
        if g == 0:
            out[b] = res.results[core]["out_ext"]
    return (out, attn), res


def kernel(**inputs):
    outputs, _ = run(trace=False, **inputs)
    return outputs


# revision 42
# speedup vs baseline: 1.7386x; 1.0249x over previous
"""Trainium2 Bass kernel for a 12-head attention block with cls-token
rebalancing (B=4, N=1024, C=768), distributed over 8 NeuronCores.

Sharding: core = 2*b + g  (b = batch 0..3, g = head-group 0..1, 6 heads each).
Each core computes qkv / attention / softmax / cls-rebalance / attn@v for its
(batch, 6 heads), plus the partial output projection over its heads' channels;
core pairs {2b, 2b+1} AllReduce the projection partials.

Outputs: attn (4,12,1024,1024) fp32 and out (4,1024,768) fp32, matching
reference.py's (out, attn) tuple.
"""

import sys

if "/opt/trn_rl_repo" not in sys.path:
    sys.path.insert(0, "/opt/trn_rl_repo")

from contextlib import ExitStack

import numpy as np

import concourse.bass as bass
import concourse.bacc as bacc
import concourse.tile as tile
from concourse import mybir
from concourse import bass_utils

F32 = mybir.dt.float32
# Matmul payload runs in bf16: fast weight load (FWL), 1 cyc/row, half the DMA.
BF16 = mybir.dt.bfloat16

B, N, C = 4, 1024, 768
H = 12
HPC = 6  # heads per core
HD = 64
SCALE = HD ** -0.5
EPS = 1e-6
NCORES = 8
REPLICA_GROUPS = [[0, 1], [2, 3], [4, 5], [6, 7]]

Exp = mybir.ActivationFunctionType.Exp
ALU = mybir.AluOpType

# bf16 packed-input column offsets (128 partitions)
OFF_xT = 0        # 6 c_in tiles x 1024 n
OFF_wqkT = 6144   # 6 c_in tiles x 768 qk cols
OFF_wvT = 10752   # 6 c_in tiles x 384 v cols
OFF_wpjT = 13056  # 3 c_in tiles x 768 cols
OFF_ones = 15360  # row (partition 0)
OFF_bv = 15488    # row
OFF_bpj = 15872   # row
PACKED = 16640
# fp32 small-constant input offsets
SOFF_ident = 0
SOFF_bqk = 128    # (128, 6)
SOFF_bvc = 134    # (128, 3)
SOFF_cls = 137    # row
SPACKED = 144


def _mm(ap):
    return ap


def build_bass():
    nc = bacc.Bacc("TRN2", debug=False, target_bir_lowering=False, num_devices=NCORES)

    # ---- external I/O: ONE packed input tensor (single DMA -> single
    # semaphore lane, since the PE LDWEIGHTS slot only fits one sync wait) ----
    inp_d = nc.dram_tensor("inp", (128, PACKED), BF16, kind="ExternalInput").ap()
    inps_d = nc.dram_tensor("inps", (128, SPACKED), F32, kind="ExternalInput").ap()

    attn_d = nc.dram_tensor("attn_out", (HPC, N, N), BF16, kind="ExternalOutput").ap()
    out_d = nc.dram_tensor("out_ext", (N, C), F32, kind="ExternalOutput").ap()
    dbg_d = None  # debug outputs disabled

    # ---- collective bounce buffers ----
    cc_in = nc.dram_tensor("cc_in", (N, C), F32).ap()
    cc_out = nc.dram_tensor("cc_out", (N, C), F32).ap()

    with tile.TileContext(nc) as tc, ExitStack() as ctx:
        P = ctx.enter_context  # pool opener

        persist = P(tc.tile_pool(name="persist", bufs=1))
        attn_pool = P(tc.tile_pool(name="attn", bufs=4))
        et_pool = P(tc.tile_pool(name="et", bufs=4))
        bc_pool = P(tc.tile_pool(name="bc", bufs=2))
        out_pool = P(tc.tile_pool(name="outsb", bufs=2))
        ps_pool = P(tc.tile_pool(name="ps", bufs=4, space="PSUM"))
        av_pool = ps_pool  # shared 4-slot rotation (8 psum banks total)

        # ---- persistent SBUF tensors ----
        inp = persist.tile([128, PACKED], BF16, tag="inp")
        inps = persist.tile([128, SPACKED], F32, tag="inps")
        xT = inp[:, OFF_xT:OFF_xT + 6 * N]
        wqkT = inp[:, OFF_wqkT:OFF_wqkT + 6 * 768]
        wvT = inp[:, OFF_wvT:OFF_wvT + 6 * 384]
        wpjT = inp[:, OFF_wpjT:OFF_wpjT + 3 * C]
        ones = inp[0:1, OFF_ones:OFF_ones + 128]
        bv = inp[0:1, OFF_bv:OFF_bv + HPC * HD]
        bpj = inp[0:1, OFF_bpj:OFF_bpj + C]
        ident = inps[:, SOFF_ident:SOFF_ident + 128]
        bqk = inps[:, SOFF_bqk:SOFF_bqk + 6]
        bvc = inps[:, SOFF_bvc:SOFF_bvc + 3]
        clsb = inps[0:1, SOFF_cls:SOFF_cls + HPC]

        qkvT = persist.tile([128, 6 * N], BF16, tag="qkvT")       # m 0..2: q pairs, 3..5: k pairs
        vsb = persist.tile([128, 8 * 384], BF16, tag="vsb")       # 8 n tiles x (6 heads*64)
        outT = persist.tile([128, 3 * N], BF16, tag="outT")       # pair j: c_in x n
        S = persist.tile([128, HPC * 8], F32, tag="S")           # row sums, head h cols h*8..
        iS = persist.tile([128, HPC * 8], F32, tag="iS")         # 1/S
        iSr = persist.tile([1, HPC * N], F32, tag="iSr")         # transposed 1/S rows
        v0T = persist.tile([128, 3], F32, tag="v0T")             # v[0,:] as columns
        cells = persist.tile([1, 8 * HPC], F32, tag="cells")     # per-head scalars

        # ---- input DMAs ----
        nc.sync.dma_start(out=inp[:, :], in_=inp_d[:, :])
        nc.sync.dma_start(out=inps[:, :], in_=inps_d[:, :])

        # ---- qk projection: qkvT[m] = (wqk tile m).T @ x, in [c_out, n] layout ----
        for m in range(6):
            ps = ps_pool.tile([128, N], F32, tag="ps")
            for nh in range(2):
                for k in range(6):
                    nc.tensor.matmul(
                        ps[:, nh * 512:(nh + 1) * 512],
                        _mm(wqkT[:, k * 768 + m * 128: k * 768 + (m + 1) * 128]),
                        _mm(xT[:, k * N + nh * 512: k * N + (nh + 1) * 512]),
                        start=(k == 0), stop=(k == 5),
                    )
            nc.vector.tensor_scalar_add(qkvT[:, m * N:(m + 1) * N], ps[:, :], bqk[:, m:m + 1])

        # ---- v projection in natural [n, d] layout: v = x.T.T @ wvT ----
        for nt in range(8):
            ps = ps_pool.tile([128, N], F32, tag="ps")
            for k in range(6):
                nc.tensor.matmul(
                    ps[:, 0:384],
                    _mm(xT[:, k * N + nt * 128: k * N + (nt + 1) * 128]),
                    _mm(wvT[:, k * 384:(k + 1) * 384]),
                    start=(k == 0), stop=False,
                )
            # + bias row via rank-1 ones matmul
            nc.tensor.matmul(ps[:, 0:384], _mm(ones[0:1, :]), _mm(bv[0:1, :]),
                             start=False, stop=True)
            nc.vector.tensor_copy(vsb[:, nt * 384:(nt + 1) * 384], ps[:, 0:384])

        # ---- v0T: v[0, :] as [d, 1] columns (for the cls row-0 output fix) ----
        for mt in range(3):
            ps = ps_pool.tile([128, N], F32, tag="ps")
            for k in range(6):
                nc.tensor.matmul(
                    ps[:, 0:1],
                    wvT[:, k * 384 + mt * 128: k * 384 + (mt + 1) * 128],
                    xT[:, k * N: k * N + 1],
                    start=(k == 0), stop=(k == 5),
                )
            nc.vector.tensor_scalar_add(v0T[:, mt:mt + 1], ps[:, 0:1], bvc[:, mt:mt + 1])

        # ---- attention per head-pair ----
        for j in range(3):
            qt_pair = qkvT[:, j * N:(j + 1) * N]
            kt_pair = qkvT[:, (3 + j) * N:(4 + j) * N]

            # pass 1: scores in [q, k] layout, softmax, cls-rebalance, attn out
            for half in range(2):
                h = 2 * j + half
                rows = slice(64 * half, 64 * half + 64)
                for qt in range(8):
                    ps = ps_pool.tile([128, N], F32, tag="ps")
                    for kh in range(2):
                        nc.tensor.matmul(
                            ps[:, kh * 512:(kh + 1) * 512],
                            _mm(qt_pair[rows, qt * 128:(qt + 1) * 128]),
                            _mm(kt_pair[rows, kh * 512:(kh + 1) * 512]),
                            start=True, stop=True,
                        )
                    at = attn_pool.tile([128, N], BF16, tag="attn")
                    sc = S[:, h * 8 + qt: h * 8 + qt + 1]
                    nc.scalar.activation(at[:, :], ps[:, :], Exp, scale=SCALE, accum_out=sc)
                    isc = iS[:, h * 8 + qt: h * 8 + qt + 1]
                    nc.vector.reciprocal(isc, sc)
                    nc.vector.tensor_scalar_mul(at[:, :], at[:, :], isc)

                    if qt == 0:
                        # cls-token rebalancing on row 0 (partition 0 of tile 0)
                        cb = cells[0:1, h * 8: h * 8 + 8]
                        a00 = at[0:1, 0:1]
                        # c0 = a00_new = min(attn00 + cls_bias, 1)
                        nc.vector.tensor_scalar(cb[0:1, 0:1], a00, clsb[0:1, h:h + 1], 1.0,
                                                op0=ALU.add, op1=ALU.min)
                        # c1 = denom = (1 + eps) - attn00   (= actual + eps)
                        nc.vector.tensor_scalar(cb[0:1, 1:2], a00, -1.0, 1.0 + EPS,
                                                op0=ALU.mult, op1=ALU.add)
                        # c2 = 1/denom
                        nc.vector.reciprocal(cb[0:1, 2:3], cb[0:1, 1:2])
                        # c3 = 1 - a00_new
                        nc.vector.tensor_scalar(cb[0:1, 3:4], cb[0:1, 0:1], -1.0, 1.0,
                                                op0=ALU.mult, op1=ALU.add)
                        # c4 = mp = (1 - a00_new) / denom
                        nc.vector.tensor_mul(cb[0:1, 4:5], cb[0:1, 3:4], cb[0:1, 2:3])
                        # c5 = mp * attn00
                        nc.vector.tensor_mul(cb[0:1, 5:6], cb[0:1, 4:5], a00)
                        # c6 = beta = a00_new - mp * attn00
                        nc.vector.tensor_sub(cb[0:1, 6:7], cb[0:1, 0:1], cb[0:1, 5:6])
                        # rescale rest of row 0 by mp, then set attn[0,0] = a00_new
                        nc.vector.tensor_scalar_mul(at[0:1, 1:N], at[0:1, 1:N], cb[0:1, 4:5])
                        nc.vector.tensor_copy(at[0:1, 0:1], cb[0:1, 0:1])

                    nc.sync.dma_start(out=attn_d[h, qt * 128:(qt + 1) * 128, :], in_=at[:, :])

                # transpose invS for this head: 8x ([128,1] -> [1,128]) on row 0
                ps = ps_pool.tile([128, N], F32, tag="ps")
                for qt in range(8):
                    nc.tensor.transpose(ps[0:1, qt * 128:(qt + 1) * 128],
                                        iS[:, h * 8 + qt: h * 8 + qt + 1], ident[:, :])
                nc.vector.tensor_copy(iSr[0:1, h * N:(h + 1) * N], ps[0:1, :])

            # pass 2: scores.T in [k, q] layout, exp, attn@v (PE col-packed pair).
            # A and B halves accumulate in SEPARATE psum tiles: hardware allows
            # only one matmul accumulation group per psum bank at a time.
            avt = [av_pool.tile([128, N], F32, name=f"avt{_h}", tag="ps") for _h in range(2)]
            for kt in range(8):
                for half in range(2):
                    h = 2 * j + half
                    rows = slice(64 * half, 64 * half + 64)
                    ps = ps_pool.tile([128, N], F32, tag="ps")
                    for qh in range(2):
                        nc.tensor.matmul(
                            ps[:, qh * 512:(qh + 1) * 512],
                            _mm(kt_pair[rows, kt * 128:(kt + 1) * 128]),
                            _mm(qt_pair[rows, qh * 512:(qh + 1) * 512]),
                            start=True, stop=True,
                        )
                    et = et_pool.tile([128, N], BF16, tag="et")
                    nc.scalar.activation(et[:, :], ps[:, :], Exp, scale=SCALE)
                    # stationary operand = both heads' v columns (128 wide);
                    # only this half's 64 output rows are meaningful, the other
                    # 64 rows of avt[half] are never read.
                    vcol = vsb[:, kt * 384 + j * 128: kt * 384 + (j + 1) * 128]
                    for qh in range(2):
                        nc.tensor.matmul(
                            avt[half][:, qh * 512:(qh + 1) * 512],
                            _mm(vcol),
                            _mm(et[:, qh * 512:(qh + 1) * 512]),
                            start=(kt == 0), stop=(kt == 7),
                        )

            # normalize attn@v by 1/S[q] (free-dim scalar via partition bcast;
            # partition_broadcast writes from partition 0, so broadcast each
            # head's row to ALL partitions and slice the half we need)
            for c in range(8):
                for half in range(2):
                    h = 2 * j + half
                    rows = slice(64 * half, 64 * half + 64)
                    bc = bc_pool.tile([128, 128], F32, name=f"bc{half}", tag="bc")
                    nc.gpsimd.partition_broadcast(
                        bc[:, :], iSr[0:1, h * N + c * 128: h * N + (c + 1) * 128])
                    nc.vector.tensor_mul(
                        outT[rows, j * N + c * 128: j * N + (c + 1) * 128],
                        avt[half][rows, c * 128:(c + 1) * 128], bc[rows, :])

            # cls row-0 output fix: out[0,:] = mp*out[0,:] + beta*v[0,:]
            for half in range(2):
                h = 2 * j + half
                rows = slice(64 * half, 64 * half + 64)
                bc = bc_pool.tile([128, 128], F32, name=f"bcf{half}", tag="bc")
                nc.gpsimd.partition_broadcast(bc[:, 0:1], cells[0:1, h * 8 + 4: h * 8 + 5])
                nc.gpsimd.partition_broadcast(bc[:, 1:2], cells[0:1, h * 8 + 6: h * 8 + 7])
                v0 = v0T[rows, j: j + 1]
                col0 = outT[rows, j * N: j * N + 1]
                # bc[:,2] = beta * v0
                nc.vector.tensor_scalar_mul(bc[rows, 2:3], v0, bc[rows, 1:2])
                # col0 = mp * col0 + beta*v0
                nc.vector.scalar_tensor_tensor(col0, col0, bc[rows, 0:1], bc[rows, 2:3],
                                               op0=ALU.mult, op1=ALU.add)

        # ---- output projection partial: out_part[n, c] over this group's c_in ----
        for nt in range(8):
            ps = ps_pool.tile([128, N], F32, tag="ps")
            for ch in range(2):
                # bank-aligned regions: [0:384] in bank 0, [512:896] in bank 1
                cs = slice(ch * 512, ch * 512 + 384)
                for ktj in range(3):
                    nc.tensor.matmul(
                        ps[:, cs],
                        _mm(outT[:, ktj * N + nt * 128: ktj * N + (nt + 1) * 128]),
                        _mm(wpjT[:, ktj * C + ch * 384: ktj * C + (ch + 1) * 384]),
                        start=(ktj == 0), stop=False,
                    )
                nc.tensor.matmul(ps[:, cs], _mm(ones[0:1, :]),
                                 _mm(bpj[0:1, ch * 384:(ch + 1) * 384]),
                                 start=False, stop=True)
            ot = out_pool.tile([128, C], F32, tag="outsb")
            nc.vector.tensor_copy(ot[:, 0:384], ps[:, 0:384])
            nc.vector.tensor_copy(ot[:, 384:768], ps[:, 512:896])
            nc.sync.dma_start(out=cc_in[nt * 128:(nt + 1) * 128, :], in_=ot[:, :])

        # ---- pair AllReduce of projection partials (4 chunks, overlapped) ----
        for cchunk in range(4):
            rs = slice(cchunk * 256, (cchunk + 1) * 256)
            nc.gpsimd.collective_compute(
                "AllReduce", ALU.add, replica_groups=REPLICA_GROUPS,
                ins=[cc_in[rs, :].opt()], outs=[cc_out[rs, :].opt()],
            )
            nc.sync.dma_start(out=out_d[rs, :], in_=cc_out[rs, :])


    nc.compile()
    _split_waits(nc)
    return nc


def _tiled_cols(a, kk):
    """(kk*128, M) -> (128, kk*M): column block k = rows k*128..(k+1)*128."""
    m = a.shape[1]
    return a.reshape(kk, 128, m).transpose(1, 0, 2).reshape(128, kk * m)


def _split_waits(nc):
    """Walrus codegen caps sync-waits at 1 per instruction (2 for
    EventSemaphore). Spill extra waits onto EventSemaphore NOPs inserted
    just before, on the same engine stream."""
    nid = [0]

    def nop_with(engine, waits):
        nid[0] += 1
        nop = mybir.InstEventSemaphore(name=f"WSPILL-{nid[0]}", ins=[], outs=[])
        nop.engine = engine
        nop.sync_info = mybir.SyncInfo(on_wait=list(waits), on_update=[])
        return nop

    for f in nc.m.functions:
        for blk in f.blocks:
            out = []
            changed = False
            for inst in blk.instructions:
                si = inst.sync_info
                waits = list(si.on_wait) if si is not None and si.on_wait else []
                cap = 2 if isinstance(inst, mybir.InstEventSemaphore) else 1
                if len(waits) > cap:
                    spill, keep = waits[:-cap], waits[-cap:]
                    for i in range(0, len(spill), 2):
                        out.append(nop_with(inst.engine, spill[i:i + 2]))
                    inst.sync_info = mybir.SyncInfo(
                        on_wait=keep, on_update=list(si.on_update) if si.on_update else [])
                    changed = True
                out.append(inst)
            if changed:
                blk.instructions = out


def make_in_maps(x, qkv_w, qkv_b, proj_w, proj_b, cls_bias):
    import ml_dtypes
    f = np.float32
    bf = ml_dtypes.bfloat16
    in_maps = []
    for core in range(NCORES):
        b, g = core // 2, core % 2
        hs = g * HPC
        qrows = qkv_w[hs * HD:(hs + HPC) * HD]            # (384, 768)
        krows = qkv_w[C + hs * HD: C + (hs + HPC) * HD]   # (384, 768)
        vrows = qkv_w[2 * C + hs * HD: 2 * C + (hs + HPC) * HD]
        bq = qkv_b[hs * HD:(hs + HPC) * HD]
        bk = qkv_b[C + hs * HD: C + (hs + HPC) * HD]
        bvv = qkv_b[2 * C + hs * HD: 2 * C + (hs + HPC) * HD]

        packed = np.zeros((128, PACKED), f)
        packed[:, OFF_xT:OFF_xT + 6 * N] = _tiled_cols(np.asarray(x[b]).T.astype(f), 6)
        packed[:, OFF_wqkT:OFF_wqkT + 6 * 768] = _tiled_cols(
            np.concatenate([qrows, krows], 0).T.astype(f), 6)
        packed[:, OFF_wvT:OFF_wvT + 6 * 384] = _tiled_cols(vrows.T.astype(f), 6)
        packed[:, OFF_wpjT:OFF_wpjT + 3 * C] = _tiled_cols(
            np.asarray(proj_w).T[hs * HD:(hs + HPC) * HD, :].astype(f), 3)
        packed[0, OFF_ones:OFF_ones + 128] = 1.0
        packed[0, OFF_bv:OFF_bv + HPC * HD] = bvv
        packed[0, OFF_bpj:OFF_bpj + C] = np.asarray(proj_b) * 0.5

        small = np.zeros((128, SPACKED), f)
        small[:, SOFF_ident:SOFF_ident + 128] = np.eye(128, dtype=f)
        small[:, SOFF_bqk:SOFF_bqk + 6] = np.concatenate([bq, bk]).reshape(6, 128).T
        small[:, SOFF_bvc:SOFF_bvc + 3] = np.asarray(bvv).reshape(3, 128).T
        small[0, SOFF_cls:SOFF_cls + HPC] = cls_bias[hs:hs + HPC]
        in_maps.append({"inp": packed.astype(bf), "inps": small})
    return in_maps


_CACHED_NC = None


def _get_nc():
    global _CACHED_NC
    if _CACHED_NC is None:
        _CACHED_NC = build_bass()
    return _CACHED_NC


def run(trace=False, **inputs):
    nc = _get_nc()
    in_maps = make_in_maps(**inputs)
    res = bass_utils.run_bass_kernel_spmd(
        nc, in_maps, core_ids=list(range(NCORES)), trace=trace,
    )
    attn = np.empty((B, H, N, N), np.float32)
    out = np.empty((B, N, C), np.float32)
    for core in range(NCORES):
        b, g = core // 2, core % 2
        attn[b, g * HPC:(g + 1) * HPC] = np.asarray(res.results[core]["attn_out"], dtype=np.float32)
        if g == 0:
            out[b] = res.results[core]["out_ext"]
    return (out, attn), res


def kernel(**inputs):
    outputs, _ = run(trace=False, **inputs)
    return outputs


# revision 43
# speedup vs baseline: 1.8984x; 1.0919x over previous
"""Trainium2 Bass kernel for a 12-head attention block with cls-token
rebalancing (B=4, N=1024, C=768), distributed over 8 NeuronCores.

Sharding: core = 2*b + g  (b = batch 0..3, g = head-group 0..1, 6 heads each).
Each core computes qkv / attention / softmax / cls-rebalance / attn@v for its
(batch, 6 heads), plus the partial output projection over its heads' channels;
core pairs {2b, 2b+1} AllReduce the projection partials.

Outputs: attn (4,12,1024,1024) fp32 and out (4,1024,768) fp32, matching
reference.py's (out, attn) tuple.
"""

import sys

if "/opt/trn_rl_repo" not in sys.path:
    sys.path.insert(0, "/opt/trn_rl_repo")

from contextlib import ExitStack

import numpy as np

import concourse.bass as bass
import concourse.bacc as bacc
import concourse.tile as tile
from concourse import mybir
from concourse import bass_utils

F32 = mybir.dt.float32
# Matmul payload runs in bf16: fast weight load (FWL), 1 cyc/row, half the DMA.
BF16 = mybir.dt.bfloat16

B, N, C = 4, 1024, 768
H = 12
HPC = 6  # heads per core
HD = 64
SCALE = HD ** -0.5
EPS = 1e-6
NCORES = 8
REPLICA_GROUPS = [[0, 1], [2, 3], [4, 5], [6, 7]]

Exp = mybir.ActivationFunctionType.Exp
ALU = mybir.AluOpType

# bf16 packed-input column offsets (128 partitions)
OFF_xT = 0        # 6 c_in tiles x 1024 n
OFF_wqkT = 6144   # 6 c_in tiles x 768 qk cols
OFF_wvT = 10752   # 6 c_in tiles x 384 v cols
OFF_wpjT = 13056  # 3 c_in tiles x 768 cols
OFF_ones = 15360  # row (partition 0)
OFF_bv = 15488    # row
OFF_bpj = 15872   # row
PACKED = 16640
# fp32 small-constant input offsets
SOFF_ident = 0
SOFF_bqk = 128    # (128, 6)
SOFF_bvc = 134    # (128, 3)
SOFF_cls = 137    # row
SPACKED = 144


def _mm(ap):
    return ap


def build_bass():
    nc = bacc.Bacc("TRN2", debug=False, target_bir_lowering=False, num_devices=NCORES)

    # ---- external I/O: ONE packed input tensor (single DMA -> single
    # semaphore lane, since the PE LDWEIGHTS slot only fits one sync wait) ----
    inp_d = nc.dram_tensor("inp", (128, PACKED), BF16, kind="ExternalInput").ap()
    inps_d = nc.dram_tensor("inps", (128, SPACKED), F32, kind="ExternalInput").ap()

    attn_d = nc.dram_tensor("attn_out", (HPC, N, N), BF16, kind="ExternalOutput").ap()
    out_d = nc.dram_tensor("out_ext", (N // 2, C), F32, kind="ExternalOutput").ap()
    dbg_d = None  # debug outputs disabled

    # ---- collective bounce buffers ----
    cc_in = nc.dram_tensor("cc_in", (N, C), F32).ap()
    cc_out = nc.dram_tensor("cc_out", (N // 2, C), F32).ap()

    with tile.TileContext(nc) as tc, ExitStack() as ctx:
        P = ctx.enter_context  # pool opener

        persist = P(tc.tile_pool(name="persist", bufs=1))
        attn_pool = P(tc.tile_pool(name="attn", bufs=4))
        et_pool = P(tc.tile_pool(name="et", bufs=4))
        bc_pool = P(tc.tile_pool(name="bc", bufs=2))
        out_pool = P(tc.tile_pool(name="outsb", bufs=2))
        ps_pool = P(tc.tile_pool(name="ps", bufs=4, space="PSUM"))
        av_pool = ps_pool  # shared 4-slot rotation (8 psum banks total)

        # ---- persistent SBUF tensors ----
        inp = persist.tile([128, PACKED], BF16, tag="inp")
        inps = persist.tile([128, SPACKED], F32, tag="inps")
        xT = inp[:, OFF_xT:OFF_xT + 6 * N]
        wqkT = inp[:, OFF_wqkT:OFF_wqkT + 6 * 768]
        wvT = inp[:, OFF_wvT:OFF_wvT + 6 * 384]
        wpjT = inp[:, OFF_wpjT:OFF_wpjT + 3 * C]
        ones = inp[0:1, OFF_ones:OFF_ones + 128]
        bv = inp[0:1, OFF_bv:OFF_bv + HPC * HD]
        bpj = inp[0:1, OFF_bpj:OFF_bpj + C]
        ident = inps[:, SOFF_ident:SOFF_ident + 128]
        bqk = inps[:, SOFF_bqk:SOFF_bqk + 6]
        bvc = inps[:, SOFF_bvc:SOFF_bvc + 3]
        clsb = inps[0:1, SOFF_cls:SOFF_cls + HPC]

        qkvT = persist.tile([128, 6 * N], BF16, tag="qkvT")       # m 0..2: q pairs, 3..5: k pairs
        vsb = persist.tile([128, 8 * 384], BF16, tag="vsb")       # 8 n tiles x (6 heads*64)
        outT = persist.tile([128, 3 * N], BF16, tag="outT")       # pair j: c_in x n
        S = persist.tile([128, HPC * 8], F32, tag="S")           # row sums, head h cols h*8..
        iS = persist.tile([128, HPC * 8], F32, tag="iS")         # 1/S
        iSr = persist.tile([1, HPC * N], F32, tag="iSr")         # transposed 1/S rows
        v0T = persist.tile([128, 3], F32, tag="v0T")             # v[0,:] as columns
        cells = persist.tile([1, 8 * HPC], F32, tag="cells")     # per-head scalars

        # ---- input DMAs ----
        nc.sync.dma_start(out=inp[:, :], in_=inp_d[:, :])
        nc.sync.dma_start(out=inps[:, :], in_=inps_d[:, :])

        # ---- qk projection: qkvT[m] = (wqk tile m).T @ x, in [c_out, n] layout ----
        for m in range(6):
            ps = ps_pool.tile([128, N], F32, tag="ps")
            for nh in range(2):
                for k in range(6):
                    nc.tensor.matmul(
                        ps[:, nh * 512:(nh + 1) * 512],
                        _mm(wqkT[:, k * 768 + m * 128: k * 768 + (m + 1) * 128]),
                        _mm(xT[:, k * N + nh * 512: k * N + (nh + 1) * 512]),
                        start=(k == 0), stop=(k == 5),
                    )
            nc.vector.tensor_scalar_add(qkvT[:, m * N:(m + 1) * N], ps[:, :], bqk[:, m:m + 1])

        # ---- v projection in natural [n, d] layout: v = x.T.T @ wvT ----
        for nt in range(8):
            ps = ps_pool.tile([128, N], F32, tag="ps")
            for k in range(6):
                nc.tensor.matmul(
                    ps[:, 0:384],
                    _mm(xT[:, k * N + nt * 128: k * N + (nt + 1) * 128]),
                    _mm(wvT[:, k * 384:(k + 1) * 384]),
                    start=(k == 0), stop=False,
                )
            # + bias row via rank-1 ones matmul
            nc.tensor.matmul(ps[:, 0:384], _mm(ones[0:1, :]), _mm(bv[0:1, :]),
                             start=False, stop=True)
            nc.vector.tensor_copy(vsb[:, nt * 384:(nt + 1) * 384], ps[:, 0:384])

        # ---- v0T: v[0, :] as [d, 1] columns (for the cls row-0 output fix) ----
        for mt in range(3):
            ps = ps_pool.tile([128, N], F32, tag="ps")
            for k in range(6):
                nc.tensor.matmul(
                    ps[:, 0:1],
                    wvT[:, k * 384 + mt * 128: k * 384 + (mt + 1) * 128],
                    xT[:, k * N: k * N + 1],
                    start=(k == 0), stop=(k == 5),
                )
            nc.vector.tensor_scalar_add(v0T[:, mt:mt + 1], ps[:, 0:1], bvc[:, mt:mt + 1])

        # ---- attention per head-pair ----
        for j in range(3):
            qt_pair = qkvT[:, j * N:(j + 1) * N]
            kt_pair = qkvT[:, (3 + j) * N:(4 + j) * N]

            # pass 1: scores in [q, k] layout, softmax, cls-rebalance, attn out
            for half in range(2):
                h = 2 * j + half
                rows = slice(64 * half, 64 * half + 64)
                for qt in range(8):
                    ps = ps_pool.tile([128, N], F32, tag="ps")
                    for kh in range(2):
                        nc.tensor.matmul(
                            ps[:, kh * 512:(kh + 1) * 512],
                            _mm(qt_pair[rows, qt * 128:(qt + 1) * 128]),
                            _mm(kt_pair[rows, kh * 512:(kh + 1) * 512]),
                            start=True, stop=True,
                        )
                    at = attn_pool.tile([128, N], BF16, tag="attn")
                    sc = S[:, h * 8 + qt: h * 8 + qt + 1]
                    nc.scalar.activation(at[:, :], ps[:, :], Exp, scale=SCALE, accum_out=sc)
                    isc = iS[:, h * 8 + qt: h * 8 + qt + 1]
                    nc.vector.reciprocal(isc, sc)
                    nc.vector.tensor_scalar_mul(at[:, :], at[:, :], isc)

                    if qt == 0:
                        # cls-token rebalancing on row 0 (partition 0 of tile 0)
                        cb = cells[0:1, h * 8: h * 8 + 8]
                        a00 = at[0:1, 0:1]
                        # c0 = a00_new = min(attn00 + cls_bias, 1)
                        nc.vector.tensor_scalar(cb[0:1, 0:1], a00, clsb[0:1, h:h + 1], 1.0,
                                                op0=ALU.add, op1=ALU.min)
                        # c1 = denom = (1 + eps) - attn00   (= actual + eps)
                        nc.vector.tensor_scalar(cb[0:1, 1:2], a00, -1.0, 1.0 + EPS,
                                                op0=ALU.mult, op1=ALU.add)
                        # c2 = 1/denom
                        nc.vector.reciprocal(cb[0:1, 2:3], cb[0:1, 1:2])
                        # c3 = 1 - a00_new
                        nc.vector.tensor_scalar(cb[0:1, 3:4], cb[0:1, 0:1], -1.0, 1.0,
                                                op0=ALU.mult, op1=ALU.add)
                        # c4 = mp = (1 - a00_new) / denom
                        nc.vector.tensor_mul(cb[0:1, 4:5], cb[0:1, 3:4], cb[0:1, 2:3])
                        # c5 = mp * attn00
                        nc.vector.tensor_mul(cb[0:1, 5:6], cb[0:1, 4:5], a00)
                        # c6 = beta = a00_new - mp * attn00
                        nc.vector.tensor_sub(cb[0:1, 6:7], cb[0:1, 0:1], cb[0:1, 5:6])
                        # rescale rest of row 0 by mp, then set attn[0,0] = a00_new
                        nc.vector.tensor_scalar_mul(at[0:1, 1:N], at[0:1, 1:N], cb[0:1, 4:5])
                        nc.vector.tensor_copy(at[0:1, 0:1], cb[0:1, 0:1])

                    nc.sync.dma_start(out=attn_d[h, qt * 128:(qt + 1) * 128, :], in_=at[:, :])

                # transpose invS for this head: 8x ([128,1] -> [1,128]) on row 0
                ps = ps_pool.tile([128, N], F32, tag="ps")
                for qt in range(8):
                    nc.tensor.transpose(ps[0:1, qt * 128:(qt + 1) * 128],
                                        iS[:, h * 8 + qt: h * 8 + qt + 1], ident[:, :])
                nc.vector.tensor_copy(iSr[0:1, h * N:(h + 1) * N], ps[0:1, :])

            # pass 2: scores.T in [k, q] layout, exp, attn@v (PE col-packed pair).
            # A and B halves accumulate in SEPARATE psum tiles: hardware allows
            # only one matmul accumulation group per psum bank at a time.
            avt = [av_pool.tile([128, N], F32, name=f"avt{_h}", tag="ps") for _h in range(2)]
            for kt in range(8):
                for half in range(2):
                    h = 2 * j + half
                    rows = slice(64 * half, 64 * half + 64)
                    ps = ps_pool.tile([128, N], F32, tag="ps")
                    for qh in range(2):
                        nc.tensor.matmul(
                            ps[:, qh * 512:(qh + 1) * 512],
                            _mm(kt_pair[rows, kt * 128:(kt + 1) * 128]),
                            _mm(qt_pair[rows, qh * 512:(qh + 1) * 512]),
                            start=True, stop=True,
                        )
                    et = et_pool.tile([128, N], BF16, tag="et")
                    nc.scalar.activation(et[:, :], ps[:, :], Exp, scale=SCALE)
                    # stationary operand = both heads' v columns (128 wide);
                    # only this half's 64 output rows are meaningful, the other
                    # 64 rows of avt[half] are never read.
                    vcol = vsb[:, kt * 384 + j * 128: kt * 384 + (j + 1) * 128]
                    for qh in range(2):
                        nc.tensor.matmul(
                            avt[half][:, qh * 512:(qh + 1) * 512],
                            _mm(vcol),
                            _mm(et[:, qh * 512:(qh + 1) * 512]),
                            start=(kt == 0), stop=(kt == 7),
                        )

            # normalize attn@v by 1/S[q] (free-dim scalar via partition bcast;
            # partition_broadcast writes from partition 0, so broadcast each
            # head's row to ALL partitions and slice the half we need)
            for c in range(8):
                for half in range(2):
                    h = 2 * j + half
                    rows = slice(64 * half, 64 * half + 64)
                    bc = bc_pool.tile([128, 128], F32, name=f"bc{half}", tag="bc")
                    nc.gpsimd.partition_broadcast(
                        bc[:, :], iSr[0:1, h * N + c * 128: h * N + (c + 1) * 128])
                    nc.vector.tensor_mul(
                        outT[rows, j * N + c * 128: j * N + (c + 1) * 128],
                        avt[half][rows, c * 128:(c + 1) * 128], bc[rows, :])

            # cls row-0 output fix: out[0,:] = mp*out[0,:] + beta*v[0,:]
            for half in range(2):
                h = 2 * j + half
                rows = slice(64 * half, 64 * half + 64)
                bc = bc_pool.tile([128, 128], F32, name=f"bcf{half}", tag="bc")
                nc.gpsimd.partition_broadcast(bc[:, 0:1], cells[0:1, h * 8 + 4: h * 8 + 5])
                nc.gpsimd.partition_broadcast(bc[:, 1:2], cells[0:1, h * 8 + 6: h * 8 + 7])
                v0 = v0T[rows, j: j + 1]
                col0 = outT[rows, j * N: j * N + 1]
                # bc[:,2] = beta * v0
                nc.vector.tensor_scalar_mul(bc[rows, 2:3], v0, bc[rows, 1:2])
                # col0 = mp * col0 + beta*v0
                nc.vector.scalar_tensor_tensor(col0, col0, bc[rows, 0:1], bc[rows, 2:3],
                                               op0=ALU.mult, op1=ALU.add)

        # ---- output projection partial: out_part[n, c] over this group's c_in ----
        for nt in range(8):
            ps = ps_pool.tile([128, N], F32, tag="ps")
            for ch in range(2):
                # bank-aligned regions: [0:384] in bank 0, [512:896] in bank 1
                cs = slice(ch * 512, ch * 512 + 384)
                for ktj in range(3):
                    nc.tensor.matmul(
                        ps[:, cs],
                        _mm(outT[:, ktj * N + nt * 128: ktj * N + (nt + 1) * 128]),
                        _mm(wpjT[:, ktj * C + ch * 384: ktj * C + (ch + 1) * 384]),
                        start=(ktj == 0), stop=False,
                    )
                nc.tensor.matmul(ps[:, cs], _mm(ones[0:1, :]),
                                 _mm(bpj[0:1, ch * 384:(ch + 1) * 384]),
                                 start=False, stop=True)
            ot = out_pool.tile([128, C], F32, tag="outsb")
            nc.vector.tensor_copy(ot[:, 0:384], ps[:, 0:384])
            nc.vector.tensor_copy(ot[:, 384:768], ps[:, 512:896])
            nc.sync.dma_start(out=cc_in[nt * 128:(nt + 1) * 128, :], in_=ot[:, :])

        # ---- pair ReduceScatter of projection partials: core 2b keeps rows
        # 0:512, core 2b+1 rows 512:1024; host concatenates. ----
        nc.gpsimd.collective_compute(
            "ReduceScatter", ALU.add, replica_groups=REPLICA_GROUPS,
            ins=[cc_in[:, :].opt()], outs=[cc_out[:, :].opt()],
        )
        nc.sync.dma_start(out=out_d[:, :], in_=cc_out[:, :])


    nc.compile()
    _split_waits(nc)
    return nc


def _tiled_cols(a, kk):
    """(kk*128, M) -> (128, kk*M): column block k = rows k*128..(k+1)*128."""
    m = a.shape[1]
    return a.reshape(kk, 128, m).transpose(1, 0, 2).reshape(128, kk * m)


def _split_waits(nc):
    """Walrus codegen caps sync-waits at 1 per instruction (2 for
    EventSemaphore). Spill extra waits onto EventSemaphore NOPs inserted
    just before, on the same engine stream."""
    nid = [0]

    def nop_with(engine, waits):
        nid[0] += 1
        nop = mybir.InstEventSemaphore(name=f"WSPILL-{nid[0]}", ins=[], outs=[])
        nop.engine = engine
        nop.sync_info = mybir.SyncInfo(on_wait=list(waits), on_update=[])
        return nop

    for f in nc.m.functions:
        for blk in f.blocks:
            out = []
            changed = False
            for inst in blk.instructions:
                si = inst.sync_info
                waits = list(si.on_wait) if si is not None and si.on_wait else []
                cap = 2 if isinstance(inst, mybir.InstEventSemaphore) else 1
                if len(waits) > cap:
                    spill, keep = waits[:-cap], waits[-cap:]
                    for i in range(0, len(spill), 2):
                        out.append(nop_with(inst.engine, spill[i:i + 2]))
                    inst.sync_info = mybir.SyncInfo(
                        on_wait=keep, on_update=list(si.on_update) if si.on_update else [])
                    changed = True
                out.append(inst)
            if changed:
                blk.instructions = out


def make_in_maps(x, qkv_w, qkv_b, proj_w, proj_b, cls_bias):
    import ml_dtypes
    f = np.float32
    bf = ml_dtypes.bfloat16
    in_maps = []
    for core in range(NCORES):
        b, g = core // 2, core % 2
        hs = g * HPC
        qrows = qkv_w[hs * HD:(hs + HPC) * HD]            # (384, 768)
        krows = qkv_w[C + hs * HD: C + (hs + HPC) * HD]   # (384, 768)
        vrows = qkv_w[2 * C + hs * HD: 2 * C + (hs + HPC) * HD]
        bq = qkv_b[hs * HD:(hs + HPC) * HD]
        bk = qkv_b[C + hs * HD: C + (hs + HPC) * HD]
        bvv = qkv_b[2 * C + hs * HD: 2 * C + (hs + HPC) * HD]

        packed = np.zeros((128, PACKED), f)
        packed[:, OFF_xT:OFF_xT + 6 * N] = _tiled_cols(np.asarray(x[b]).T.astype(f), 6)
        packed[:, OFF_wqkT:OFF_wqkT + 6 * 768] = _tiled_cols(
            np.concatenate([qrows, krows], 0).T.astype(f), 6)
        packed[:, OFF_wvT:OFF_wvT + 6 * 384] = _tiled_cols(vrows.T.astype(f), 6)
        packed[:, OFF_wpjT:OFF_wpjT + 3 * C] = _tiled_cols(
            np.asarray(proj_w).T[hs * HD:(hs + HPC) * HD, :].astype(f), 3)
        packed[0, OFF_ones:OFF_ones + 128] = 1.0
        packed[0, OFF_bv:OFF_bv + HPC * HD] = bvv
        packed[0, OFF_bpj:OFF_bpj + C] = np.asarray(proj_b) * 0.5

        small = np.zeros((128, SPACKED), f)
        small[:, SOFF_ident:SOFF_ident + 128] = np.eye(128, dtype=f)
        small[:, SOFF_bqk:SOFF_bqk + 6] = np.concatenate([bq, bk]).reshape(6, 128).T
        small[:, SOFF_bvc:SOFF_bvc + 3] = np.asarray(bvv).reshape(3, 128).T
        small[0, SOFF_cls:SOFF_cls + HPC] = cls_bias[hs:hs + HPC]
        in_maps.append({"inp": packed.astype(bf), "inps": small})
    return in_maps


_CACHED_NC = None


def _get_nc():
    global _CACHED_NC
    if _CACHED_NC is None:
        _CACHED_NC = build_bass()
    return _CACHED_NC


def run(trace=False, **inputs):
    nc = _get_nc()
    in_maps = make_in_maps(**inputs)
    res = bass_utils.run_bass_kernel_spmd(
        nc, in_maps, core_ids=list(range(NCORES)), trace=trace,
    )
    attn = np.empty((B, H, N, N), np.float32)
    out = np.empty((B, N, C), np.float32)
    for core in range(NCORES):
        b, g = core // 2, core % 2
        attn[b, g * HPC:(g + 1) * HPC] = np.asarray(res.results[core]["attn_out"], dtype=np.float32)
        out[b, g * (N // 2):(g + 1) * (N // 2)] = res.results[core]["out_ext"]
    return (out, attn), res


def kernel(**inputs):
    outputs, _ = run(trace=False, **inputs)
    return outputs


# revision 44
# speedup vs baseline: 2.0433x; 1.0763x over previous
"""Trainium2 Bass kernel for a 12-head attention block with cls-token
rebalancing (B=4, N=1024, C=768), distributed over 8 NeuronCores.

Sharding: core = 2*b + g  (b = batch 0..3, g = head-group 0..1, 6 heads each).
Each core computes qkv / attention / softmax / cls-rebalance / attn@v for its
(batch, 6 heads), plus the partial output projection over its heads' channels;
core pairs {2b, 2b+1} AllReduce the projection partials.

Outputs: attn (4,12,1024,1024) fp32 and out (4,1024,768) fp32, matching
reference.py's (out, attn) tuple.
"""

import sys

if "/opt/trn_rl_repo" not in sys.path:
    sys.path.insert(0, "/opt/trn_rl_repo")

from contextlib import ExitStack

import numpy as np

import concourse.bass as bass
import concourse.bacc as bacc
import concourse.tile as tile
from concourse import mybir
from concourse import bass_utils

F32 = mybir.dt.float32
# Matmul payload runs in bf16: fast weight load (FWL), 1 cyc/row, half the DMA.
BF16 = mybir.dt.bfloat16

B, N, C = 4, 1024, 768
H = 12
HPC = 6  # heads per core
HD = 64
SCALE = HD ** -0.5
EPS = 1e-6
NCORES = 8
REPLICA_GROUPS = [[0, 1], [2, 3], [4, 5], [6, 7]]

Exp = mybir.ActivationFunctionType.Exp
ALU = mybir.AluOpType

# bf16 packed-input column offsets (128 partitions)
OFF_xT = 0        # 6 c_in tiles x 1024 n
OFF_wqkT = 6144   # 6 c_in tiles x 768 qk cols
OFF_wvT = 10752   # 6 c_in tiles x 384 v cols
OFF_wpjT = 13056  # 3 c_in tiles x 768 cols
OFF_ones = 15360  # row (partition 0)
OFF_bv = 15488    # row
OFF_bpj = 15872   # row
PACKED = 16640
# fp32 small-constant input offsets
SOFF_ident = 0
SOFF_bqk = 128    # (128, 6)
SOFF_bvc = 134    # (128, 3)
SOFF_cls = 137    # row
SPACKED = 144


def _mm(ap):
    return ap


def build_bass():
    nc = bacc.Bacc("TRN2", debug=False, target_bir_lowering=False, num_devices=NCORES)

    # ---- external I/O: ONE packed input tensor (single DMA -> single
    # semaphore lane, since the PE LDWEIGHTS slot only fits one sync wait) ----
    inp_d = nc.dram_tensor("inp", (128, PACKED), BF16, kind="ExternalInput").ap()
    inps_d = nc.dram_tensor("inps", (128, SPACKED), F32, kind="ExternalInput").ap()

    attn_d = nc.dram_tensor("attn_out", (HPC, N, N), BF16, kind="ExternalOutput").ap()
    out_d = nc.dram_tensor("out_ext", (N // 2, C), BF16, kind="ExternalOutput").ap()
    dbg_d = None  # debug outputs disabled

    # ---- collective bounce buffers ----
    cc_in = nc.dram_tensor("cc_in", (N, C), BF16).ap()
    cc_out = nc.dram_tensor("cc_out", (N // 2, C), BF16).ap()

    with tile.TileContext(nc) as tc, ExitStack() as ctx:
        P = ctx.enter_context  # pool opener

        persist = P(tc.tile_pool(name="persist", bufs=1))
        attn_pool = P(tc.tile_pool(name="attn", bufs=4))
        et_pool = P(tc.tile_pool(name="et", bufs=4))
        bc_pool = P(tc.tile_pool(name="bc", bufs=2))
        out_pool = P(tc.tile_pool(name="outsb", bufs=2))
        ps_pool = P(tc.tile_pool(name="ps", bufs=4, space="PSUM"))
        av_pool = ps_pool  # shared 4-slot rotation (8 psum banks total)

        # ---- persistent SBUF tensors ----
        inp = persist.tile([128, PACKED], BF16, tag="inp")
        inps = persist.tile([128, SPACKED], F32, tag="inps")
        xT = inp[:, OFF_xT:OFF_xT + 6 * N]
        wqkT = inp[:, OFF_wqkT:OFF_wqkT + 6 * 768]
        wvT = inp[:, OFF_wvT:OFF_wvT + 6 * 384]
        wpjT = inp[:, OFF_wpjT:OFF_wpjT + 3 * C]
        ones = inp[0:1, OFF_ones:OFF_ones + 128]
        bv = inp[0:1, OFF_bv:OFF_bv + HPC * HD]
        bpj = inp[0:1, OFF_bpj:OFF_bpj + C]
        ident = inps[:, SOFF_ident:SOFF_ident + 128]
        bqk = inps[:, SOFF_bqk:SOFF_bqk + 6]
        bvc = inps[:, SOFF_bvc:SOFF_bvc + 3]
        clsb = inps[0:1, SOFF_cls:SOFF_cls + HPC]

        qkvT = persist.tile([128, 6 * N], BF16, tag="qkvT")       # m 0..2: q pairs, 3..5: k pairs
        vsb = persist.tile([128, 8 * 384], BF16, tag="vsb")       # 8 n tiles x (6 heads*64)
        outT = persist.tile([128, 3 * N], BF16, tag="outT")       # pair j: c_in x n
        S = persist.tile([128, HPC * 8], F32, tag="S")           # row sums, head h cols h*8..
        iS = persist.tile([128, HPC * 8], F32, tag="iS")         # 1/S
        iSr = persist.tile([1, HPC * N], F32, tag="iSr")         # transposed 1/S rows
        v0T = persist.tile([128, 3], F32, tag="v0T")             # v[0,:] as columns
        cells = persist.tile([1, 8 * HPC], F32, tag="cells")     # per-head scalars

        # ---- input DMAs ----
        nc.sync.dma_start(out=inp[:, :], in_=inp_d[:, :])
        nc.sync.dma_start(out=inps[:, :], in_=inps_d[:, :])

        def emit_qkv(m):
            ps = ps_pool.tile([128, N], F32, name=f"qkvps{m}", tag="ps")
            for nh in range(2):
                for k in range(6):
                    nc.tensor.matmul(
                        ps[:, nh * 512:(nh + 1) * 512],
                        _mm(wqkT[:, k * 768 + m * 128: k * 768 + (m + 1) * 128]),
                        _mm(xT[:, k * N + nh * 512: k * N + (nh + 1) * 512]),
                        start=(k == 0), stop=(k == 5),
                    )
            nc.vector.tensor_scalar_add(qkvT[:, m * N:(m + 1) * N], ps[:, :], bqk[:, m:m + 1])

        def emit_v():
            for nt in range(8):
                ps = ps_pool.tile([128, N], F32, name=f"vps{nt}", tag="ps")
                for k in range(6):
                    nc.tensor.matmul(
                        ps[:, 0:384],
                        _mm(xT[:, k * N + nt * 128: k * N + (nt + 1) * 128]),
                        _mm(wvT[:, k * 384:(k + 1) * 384]),
                        start=(k == 0), stop=False,
                    )
                nc.tensor.matmul(ps[:, 0:384], _mm(ones[0:1, :]), _mm(bv[0:1, :]),
                                 start=False, stop=True)
                nc.vector.tensor_copy(vsb[:, nt * 384:(nt + 1) * 384], ps[:, 0:384])
            for mt in range(3):
                ps = ps_pool.tile([128, N], F32, name=f"v0ps{mt}", tag="ps")
                for k in range(6):
                    nc.tensor.matmul(
                        ps[:, 0:1],
                        wvT[:, k * 384 + mt * 128: k * 384 + (mt + 1) * 128],
                        xT[:, k * N: k * N + 1],
                        start=(k == 0), stop=(k == 5),
                    )
                nc.vector.tensor_scalar_add(v0T[:, mt:mt + 1], ps[:, 0:1], bvc[:, mt:mt + 1])

        def emit_pass1(j):
            qt_pair = qkvT[:, j * N:(j + 1) * N]
            kt_pair = qkvT[:, (3 + j) * N:(4 + j) * N]
            for half in range(2):
                h = 2 * j + half
                rows = slice(64 * half, 64 * half + 64)
                for qt in range(8):
                    ps = ps_pool.tile([128, N], F32, name=f"s1_{h}_{qt}", tag="ps")
                    for kh in range(2):
                        nc.tensor.matmul(
                            ps[:, kh * 512:(kh + 1) * 512],
                            _mm(qt_pair[rows, qt * 128:(qt + 1) * 128]),
                            _mm(kt_pair[rows, kh * 512:(kh + 1) * 512]),
                            start=True, stop=True,
                        )
                    at = attn_pool.tile([128, N], BF16, name=f"at{h}_{qt}", tag="attn")
                    sc = S[:, h * 8 + qt: h * 8 + qt + 1]
                    nc.scalar.activation(at[:, :], ps[:, :], Exp, scale=SCALE, accum_out=sc)
                    isc = iS[:, h * 8 + qt: h * 8 + qt + 1]
                    nc.vector.reciprocal(isc, sc)
                    nc.vector.tensor_scalar_mul(at[:, :], at[:, :], isc)

                    if qt == 0:
                        cb = cells[0:1, h * 8: h * 8 + 8]
                        a00 = at[0:1, 0:1]
                        nc.vector.tensor_scalar(cb[0:1, 0:1], a00, clsb[0:1, h:h + 1], 1.0,
                                                op0=ALU.add, op1=ALU.min)
                        nc.vector.tensor_scalar(cb[0:1, 1:2], a00, -1.0, 1.0 + EPS,
                                                op0=ALU.mult, op1=ALU.add)
                        nc.vector.reciprocal(cb[0:1, 2:3], cb[0:1, 1:2])
                        nc.vector.tensor_scalar(cb[0:1, 3:4], cb[0:1, 0:1], -1.0, 1.0,
                                                op0=ALU.mult, op1=ALU.add)
                        nc.vector.tensor_mul(cb[0:1, 4:5], cb[0:1, 3:4], cb[0:1, 2:3])
                        nc.vector.tensor_mul(cb[0:1, 5:6], cb[0:1, 4:5], a00)
                        nc.vector.tensor_sub(cb[0:1, 6:7], cb[0:1, 0:1], cb[0:1, 5:6])
                        nc.vector.tensor_scalar_mul(at[0:1, 1:N], at[0:1, 1:N], cb[0:1, 4:5])
                        nc.vector.tensor_copy(at[0:1, 0:1], cb[0:1, 0:1])

                    nc.sync.dma_start(out=attn_d[h, qt * 128:(qt + 1) * 128, :], in_=at[:, :])

                ps = ps_pool.tile([128, N], F32, name=f"ivt{h}", tag="ps")
                for qt in range(8):
                    nc.tensor.transpose(ps[0:1, qt * 128:(qt + 1) * 128],
                                        iS[:, h * 8 + qt: h * 8 + qt + 1], ident[:, :])
                nc.vector.tensor_copy(iSr[0:1, h * N:(h + 1) * N], ps[0:1, :])

        def emit_pass2(j):
            qt_pair = qkvT[:, j * N:(j + 1) * N]
            kt_pair = qkvT[:, (3 + j) * N:(4 + j) * N]
            avt = [ps_pool.tile([128, N], F32, name=f"avt{j}_{_h}", tag="ps") for _h in range(2)]
            for kt in range(8):
                for half in range(2):
                    h = 2 * j + half
                    rows = slice(64 * half, 64 * half + 64)
                    ps = ps_pool.tile([128, N], F32, name=f"s2_{h}_{kt}", tag="ps")
                    for qh in range(2):
                        nc.tensor.matmul(
                            ps[:, qh * 512:(qh + 1) * 512],
                            _mm(kt_pair[rows, kt * 128:(kt + 1) * 128]),
                            _mm(qt_pair[rows, qh * 512:(qh + 1) * 512]),
                            start=True, stop=True,
                        )
                    et = et_pool.tile([128, N], BF16, name=f"et{h}_{kt}", tag="et")
                    nc.scalar.activation(et[:, :], ps[:, :], Exp, scale=SCALE)
                    vcol = vsb[:, kt * 384 + j * 128: kt * 384 + (j + 1) * 128]
                    for qh in range(2):
                        nc.tensor.matmul(
                            avt[half][:, qh * 512:(qh + 1) * 512],
                            _mm(vcol),
                            _mm(et[:, qh * 512:(qh + 1) * 512]),
                            start=(kt == 0), stop=(kt == 7),
                        )
            for c in range(8):
                for half in range(2):
                    h = 2 * j + half
                    rows = slice(64 * half, 64 * half + 64)
                    bc = bc_pool.tile([128, 128], F32, name=f"bc{half}", tag="bc")
                    nc.gpsimd.partition_broadcast(
                        bc[:, :], iSr[0:1, h * N + c * 128: h * N + (c + 1) * 128])
                    nc.vector.tensor_mul(
                        outT[rows, j * N + c * 128: j * N + (c + 1) * 128],
                        avt[half][rows, c * 128:(c + 1) * 128], bc[rows, :])
            for half in range(2):
                h = 2 * j + half
                rows = slice(64 * half, 64 * half + 64)
                bc = bc_pool.tile([128, 128], F32, name=f"bcf{half}", tag="bc")
                nc.gpsimd.partition_broadcast(bc[:, 0:1], cells[0:1, h * 8 + 4: h * 8 + 5])
                nc.gpsimd.partition_broadcast(bc[:, 1:2], cells[0:1, h * 8 + 6: h * 8 + 7])
                v0 = v0T[rows, j: j + 1]
                col0 = outT[rows, j * N: j * N + 1]
                nc.vector.tensor_scalar_mul(bc[rows, 2:3], v0, bc[rows, 1:2])
                nc.vector.scalar_tensor_tensor(col0, col0, bc[rows, 0:1], bc[rows, 2:3],
                                               op0=ALU.mult, op1=ALU.add)

        # emission order: get pair-0 softmax running ASAP, fill PE with v/qkv later
        emit_qkv(0); emit_qkv(3)
        emit_pass1(0)
        emit_qkv(1); emit_qkv(4)
        emit_v()
        emit_pass2(0)
        emit_pass1(1)
        emit_qkv(2); emit_qkv(5)
        emit_pass2(1)
        emit_pass1(2)
        emit_pass2(2)

        # ---- output projection partial: out_part[n, c] over this group's c_in ----
        for nt in range(8):
            ps = ps_pool.tile([128, N], F32, tag="ps")
            for ch in range(2):
                # bank-aligned regions: [0:384] in bank 0, [512:896] in bank 1
                cs = slice(ch * 512, ch * 512 + 384)
                for ktj in range(3):
                    nc.tensor.matmul(
                        ps[:, cs],
                        _mm(outT[:, ktj * N + nt * 128: ktj * N + (nt + 1) * 128]),
                        _mm(wpjT[:, ktj * C + ch * 384: ktj * C + (ch + 1) * 384]),
                        start=(ktj == 0), stop=False,
                    )
                nc.tensor.matmul(ps[:, cs], _mm(ones[0:1, :]),
                                 _mm(bpj[0:1, ch * 384:(ch + 1) * 384]),
                                 start=False, stop=True)
            ot = out_pool.tile([128, C], BF16, tag="outsb")
            nc.vector.tensor_copy(ot[:, 0:384], ps[:, 0:384])
            nc.vector.tensor_copy(ot[:, 384:768], ps[:, 512:896])
            nc.sync.dma_start(out=cc_in[nt * 128:(nt + 1) * 128, :], in_=ot[:, :])

        # ---- pair ReduceScatter of projection partials: core 2b keeps rows
        # 0:512, core 2b+1 rows 512:1024; host concatenates. ----
        nc.gpsimd.collective_compute(
            "ReduceScatter", ALU.add, replica_groups=REPLICA_GROUPS,
            ins=[cc_in[:, :].opt()], outs=[cc_out[:, :].opt()],
        )
        nc.sync.dma_start(out=out_d[:, :], in_=cc_out[:, :])


    nc.compile()
    _split_waits(nc)
    return nc


def _tiled_cols(a, kk):
    """(kk*128, M) -> (128, kk*M): column block k = rows k*128..(k+1)*128."""
    m = a.shape[1]
    return a.reshape(kk, 128, m).transpose(1, 0, 2).reshape(128, kk * m)


def _split_waits(nc):
    """Walrus codegen caps sync-waits at 1 per instruction (2 for
    EventSemaphore). Spill extra waits onto EventSemaphore NOPs inserted
    just before, on the same engine stream."""
    nid = [0]

    def nop_with(engine, waits):
        nid[0] += 1
        nop = mybir.InstEventSemaphore(name=f"WSPILL-{nid[0]}", ins=[], outs=[])
        nop.engine = engine
        nop.sync_info = mybir.SyncInfo(on_wait=list(waits), on_update=[])
        return nop

    for f in nc.m.functions:
        for blk in f.blocks:
            out = []
            changed = False
            for inst in blk.instructions:
                si = inst.sync_info
                waits = list(si.on_wait) if si is not None and si.on_wait else []
                cap = 2 if isinstance(inst, mybir.InstEventSemaphore) else 1
                if len(waits) > cap:
                    spill, keep = waits[:-cap], waits[-cap:]
                    for i in range(0, len(spill), 2):
                        out.append(nop_with(inst.engine, spill[i:i + 2]))
                    inst.sync_info = mybir.SyncInfo(
                        on_wait=keep, on_update=list(si.on_update) if si.on_update else [])
                    changed = True
                out.append(inst)
            if changed:
                blk.instructions = out


def make_in_maps(x, qkv_w, qkv_b, proj_w, proj_b, cls_bias):
    import ml_dtypes
    f = np.float32
    bf = ml_dtypes.bfloat16
    in_maps = []
    for core in range(NCORES):
        b, g = core // 2, core % 2
        hs = g * HPC
        qrows = qkv_w[hs * HD:(hs + HPC) * HD]            # (384, 768)
        krows = qkv_w[C + hs * HD: C + (hs + HPC) * HD]   # (384, 768)
        vrows = qkv_w[2 * C + hs * HD: 2 * C + (hs + HPC) * HD]
        bq = qkv_b[hs * HD:(hs + HPC) * HD]
        bk = qkv_b[C + hs * HD: C + (hs + HPC) * HD]
        bvv = qkv_b[2 * C + hs * HD: 2 * C + (hs + HPC) * HD]

        packed = np.zeros((128, PACKED), f)
        packed[:, OFF_xT:OFF_xT + 6 * N] = _tiled_cols(np.asarray(x[b]).T.astype(f), 6)
        packed[:, OFF_wqkT:OFF_wqkT + 6 * 768] = _tiled_cols(
            np.concatenate([qrows, krows], 0).T.astype(f), 6)
        packed[:, OFF_wvT:OFF_wvT + 6 * 384] = _tiled_cols(vrows.T.astype(f), 6)
        packed[:, OFF_wpjT:OFF_wpjT + 3 * C] = _tiled_cols(
            np.asarray(proj_w).T[hs * HD:(hs + HPC) * HD, :].astype(f), 3)
        packed[0, OFF_ones:OFF_ones + 128] = 1.0
        packed[0, OFF_bv:OFF_bv + HPC * HD] = bvv
        packed[0, OFF_bpj:OFF_bpj + C] = np.asarray(proj_b) * 0.5

        small = np.zeros((128, SPACKED), f)
        small[:, SOFF_ident:SOFF_ident + 128] = np.eye(128, dtype=f)
        small[:, SOFF_bqk:SOFF_bqk + 6] = np.concatenate([bq, bk]).reshape(6, 128).T
        small[:, SOFF_bvc:SOFF_bvc + 3] = np.asarray(bvv).reshape(3, 128).T
        small[0, SOFF_cls:SOFF_cls + HPC] = cls_bias[hs:hs + HPC]
        in_maps.append({"inp": packed.astype(bf), "inps": small})
    return in_maps


_CACHED_NC = None


def _get_nc():
    global _CACHED_NC
    if _CACHED_NC is None:
        _CACHED_NC = build_bass()
    return _CACHED_NC


def run(trace=False, **inputs):
    nc = _get_nc()
    in_maps = make_in_maps(**inputs)
    res = bass_utils.run_bass_kernel_spmd(
        nc, in_maps, core_ids=list(range(NCORES)), trace=trace,
    )
    attn = np.empty((B, H, N, N), np.float32)
    out = np.empty((B, N, C), np.float32)
    for core in range(NCORES):
        b, g = core // 2, core % 2
        attn[b, g * HPC:(g + 1) * HPC] = np.asarray(res.results[core]["attn_out"], dtype=np.float32)
        out[b, g * (N // 2):(g + 1) * (N // 2)] = np.asarray(res.results[core]["out_ext"], dtype=np.float32)
    return (out, attn), res


def kernel(**inputs):
    outputs, _ = run(trace=False, **inputs)
    return outputs


# revision 45
# speedup vs baseline: 2.1466x; 1.0506x over previous
"""Trainium2 Bass kernel for a 12-head attention block with cls-token
rebalancing (B=4, N=1024, C=768), distributed over 8 NeuronCores.

Sharding: core = 2*b + g  (b = batch 0..3, g = head-group 0..1, 6 heads each).
Each core computes qkv / attention / softmax / cls-rebalance / attn@v for its
(batch, 6 heads), plus the partial output projection over its heads' channels;
core pairs {2b, 2b+1} AllReduce the projection partials.

Outputs: attn (4,12,1024,1024) fp32 and out (4,1024,768) fp32, matching
reference.py's (out, attn) tuple.
"""

import sys

if "/opt/trn_rl_repo" not in sys.path:
    sys.path.insert(0, "/opt/trn_rl_repo")

from contextlib import ExitStack

import numpy as np

import concourse.bass as bass
import concourse.bacc as bacc
import concourse.tile as tile
from concourse import mybir
from concourse import bass_utils

F32 = mybir.dt.float32
# Matmul payload runs in bf16: fast weight load (FWL), 1 cyc/row, half the DMA.
BF16 = mybir.dt.bfloat16

B, N, C = 4, 1024, 768
H = 12
HPC = 6  # heads per core
HD = 64
SCALE = HD ** -0.5
EPS = 1e-6
NCORES = 8
REPLICA_GROUPS = [[0, 1], [2, 3], [4, 5], [6, 7]]

Exp = mybir.ActivationFunctionType.Exp
ALU = mybir.AluOpType

# bf16 packed-input column offsets (128 partitions)
OFF_xT = 0        # 6 c_in tiles x 1024 n
OFF_wqkT = 6144   # 6 c_in tiles x 768 qk cols
OFF_wvT = 10752   # 6 c_in tiles x 384 v cols
OFF_wpjT = 13056  # 3 c_in tiles x 768 cols
OFF_ones = 15360  # row (partition 0)
OFF_bv = 15488    # row
OFF_bpj = 15872   # row
PACKED = 16640
# fp32 small-constant input offsets
SOFF_ident = 0
SOFF_bqk = 128    # (128, 6)
SOFF_bvc = 134    # (128, 3)
SOFF_cls = 137    # row
SPACKED = 144


def _mm(ap):
    return ap


def build_bass():
    nc = bacc.Bacc("TRN2", debug=False, target_bir_lowering=False, num_devices=NCORES)

    # ---- external I/O: ONE packed input tensor (single DMA -> single
    # semaphore lane, since the PE LDWEIGHTS slot only fits one sync wait) ----
    inp_d = nc.dram_tensor("inp", (128, PACKED), BF16, kind="ExternalInput").ap()
    inps_d = nc.dram_tensor("inps", (128, SPACKED), F32, kind="ExternalInput").ap()

    attn_d = nc.dram_tensor("attn_out", (HPC, N, N), BF16, kind="ExternalOutput").ap()
    out_d = nc.dram_tensor("out_ext", (N // 2, C), BF16, kind="ExternalOutput").ap()
    dbg_d = None  # debug outputs disabled

    # ---- collective bounce buffers ----
    cc_in = nc.dram_tensor("cc_in", (N, C), BF16).ap()
    cc_out = nc.dram_tensor("cc_out", (N // 2, C), BF16).ap()

    with tile.TileContext(nc) as tc, ExitStack() as ctx:
        P = ctx.enter_context  # pool opener

        persist = P(tc.tile_pool(name="persist", bufs=1))
        attn_pool = P(tc.tile_pool(name="attn", bufs=6))
        et_pool = P(tc.tile_pool(name="et", bufs=6))
        bc_pool = P(tc.tile_pool(name="bc", bufs=4))
        out_pool = P(tc.tile_pool(name="outsb", bufs=2))
        ps_pool = P(tc.tile_pool(name="ps", bufs=4, space="PSUM"))
        av_pool = ps_pool  # shared 4-slot rotation (8 psum banks total)

        # ---- persistent SBUF tensors ----
        inp = persist.tile([128, PACKED], BF16, tag="inp")
        inps = persist.tile([128, SPACKED], F32, tag="inps")
        xT = inp[:, OFF_xT:OFF_xT + 6 * N]
        wqkT = inp[:, OFF_wqkT:OFF_wqkT + 6 * 768]
        wvT = inp[:, OFF_wvT:OFF_wvT + 6 * 384]
        wpjT = inp[:, OFF_wpjT:OFF_wpjT + 3 * C]
        ones = inp[0:1, OFF_ones:OFF_ones + 128]
        bv = inp[0:1, OFF_bv:OFF_bv + HPC * HD]
        bpj = inp[0:1, OFF_bpj:OFF_bpj + C]
        ident = inps[:, SOFF_ident:SOFF_ident + 128]
        bqk = inps[:, SOFF_bqk:SOFF_bqk + 6]
        bvc = inps[:, SOFF_bvc:SOFF_bvc + 3]
        clsb = inps[0:1, SOFF_cls:SOFF_cls + HPC]

        qkvT = persist.tile([128, 6 * N], BF16, tag="qkvT")       # m 0..2: q pairs, 3..5: k pairs
        vsb = persist.tile([128, 8 * 384], BF16, tag="vsb")       # 8 n tiles x (6 heads*64)
        outT = persist.tile([128, 3 * N], BF16, tag="outT")       # pair j: c_in x n
        S = persist.tile([128, HPC * 8], F32, tag="S")           # row sums, head h cols h*8..
        iS = persist.tile([128, HPC * 8], F32, tag="iS")         # 1/S
        iSr = persist.tile([1, HPC * N], F32, tag="iSr")         # transposed 1/S rows
        v0T = persist.tile([128, 3], F32, tag="v0T")             # v[0,:] as columns
        cells = persist.tile([1, 8 * HPC], F32, tag="cells")     # per-head scalars

        # ---- input DMAs ----
        nc.sync.dma_start(out=inp[:, :], in_=inp_d[:, :])
        nc.sync.dma_start(out=inps[:, :], in_=inps_d[:, :])

        def emit_qkv(m):
            ps = ps_pool.tile([128, N], F32, name=f"qkvps{m}", tag="ps")
            for nh in range(2):
                for k in range(6):
                    nc.tensor.matmul(
                        ps[:, nh * 512:(nh + 1) * 512],
                        _mm(wqkT[:, k * 768 + m * 128: k * 768 + (m + 1) * 128]),
                        _mm(xT[:, k * N + nh * 512: k * N + (nh + 1) * 512]),
                        start=(k == 0), stop=(k == 5),
                    )
            nc.vector.tensor_scalar_add(qkvT[:, m * N:(m + 1) * N], ps[:, :], bqk[:, m:m + 1])

        def emit_v():
            for nt in range(8):
                ps = ps_pool.tile([128, N], F32, name=f"vps{nt}", tag="ps")
                for k in range(6):
                    nc.tensor.matmul(
                        ps[:, 0:384],
                        _mm(xT[:, k * N + nt * 128: k * N + (nt + 1) * 128]),
                        _mm(wvT[:, k * 384:(k + 1) * 384]),
                        start=(k == 0), stop=False,
                    )
                nc.tensor.matmul(ps[:, 0:384], _mm(ones[0:1, :]), _mm(bv[0:1, :]),
                                 start=False, stop=True)
                nc.vector.tensor_copy(vsb[:, nt * 384:(nt + 1) * 384], ps[:, 0:384])
            for mt in range(3):
                ps = ps_pool.tile([128, N], F32, name=f"v0ps{mt}", tag="ps")
                for k in range(6):
                    nc.tensor.matmul(
                        ps[:, 0:1],
                        wvT[:, k * 384 + mt * 128: k * 384 + (mt + 1) * 128],
                        xT[:, k * N: k * N + 1],
                        start=(k == 0), stop=(k == 5),
                    )
                nc.vector.tensor_scalar_add(v0T[:, mt:mt + 1], ps[:, 0:1], bvc[:, mt:mt + 1])

        def emit_pass1(j):
            qt_pair = qkvT[:, j * N:(j + 1) * N]
            kt_pair = qkvT[:, (3 + j) * N:(4 + j) * N]
            for half in range(2):
                h = 2 * j + half
                rows = slice(64 * half, 64 * half + 64)
                for qt in range(8):
                    ps = ps_pool.tile([128, N], F32, name=f"s1_{h}_{qt}", tag="ps")
                    for kh in range(2):
                        nc.tensor.matmul(
                            ps[:, kh * 512:(kh + 1) * 512],
                            _mm(qt_pair[rows, qt * 128:(qt + 1) * 128]),
                            _mm(kt_pair[rows, kh * 512:(kh + 1) * 512]),
                            start=True, stop=True,
                        )
                    at = attn_pool.tile([128, N], BF16, name=f"at{h}_{qt}", tag="attn")
                    sc = S[:, h * 8 + qt: h * 8 + qt + 1]
                    nc.scalar.activation(at[:, :], ps[:, :], Exp, scale=SCALE, accum_out=sc)
                    isc = iS[:, h * 8 + qt: h * 8 + qt + 1]
                    nc.vector.reciprocal(isc, sc)
                    nc.vector.tensor_scalar_mul(at[:, :], at[:, :], isc)

                    if qt == 0:
                        cb = cells[0:1, h * 8: h * 8 + 8]
                        a00 = at[0:1, 0:1]
                        nc.vector.tensor_scalar(cb[0:1, 0:1], a00, clsb[0:1, h:h + 1], 1.0,
                                                op0=ALU.add, op1=ALU.min)
                        nc.vector.tensor_scalar(cb[0:1, 1:2], a00, -1.0, 1.0 + EPS,
                                                op0=ALU.mult, op1=ALU.add)
                        nc.vector.reciprocal(cb[0:1, 2:3], cb[0:1, 1:2])
                        nc.vector.tensor_scalar(cb[0:1, 3:4], cb[0:1, 0:1], -1.0, 1.0,
                                                op0=ALU.mult, op1=ALU.add)
                        nc.vector.tensor_mul(cb[0:1, 4:5], cb[0:1, 3:4], cb[0:1, 2:3])
                        nc.vector.tensor_mul(cb[0:1, 5:6], cb[0:1, 4:5], a00)
                        nc.vector.tensor_sub(cb[0:1, 6:7], cb[0:1, 0:1], cb[0:1, 5:6])
                        nc.vector.tensor_scalar_mul(at[0:1, 1:N], at[0:1, 1:N], cb[0:1, 4:5])
                        nc.vector.tensor_copy(at[0:1, 0:1], cb[0:1, 0:1])

                    nc.sync.dma_start(out=attn_d[h, qt * 128:(qt + 1) * 128, :], in_=at[:, :])

                ps = ps_pool.tile([128, N], F32, name=f"ivt{h}", tag="ps")
                for qt in range(8):
                    nc.tensor.transpose(ps[0:1, qt * 128:(qt + 1) * 128],
                                        iS[:, h * 8 + qt: h * 8 + qt + 1], ident[:, :])
                nc.vector.tensor_copy(iSr[0:1, h * N:(h + 1) * N], ps[0:1, :])

        def emit_pass2(j):
            qt_pair = qkvT[:, j * N:(j + 1) * N]
            kt_pair = qkvT[:, (3 + j) * N:(4 + j) * N]
            avt = [ps_pool.tile([128, N], F32, name=f"avt{j}_{_h}", tag="ps") for _h in range(2)]
            for kt in range(8):
                for half in range(2):
                    h = 2 * j + half
                    rows = slice(64 * half, 64 * half + 64)
                    ps = ps_pool.tile([128, N], F32, name=f"s2_{h}_{kt}", tag="ps")
                    for qh in range(2):
                        nc.tensor.matmul(
                            ps[:, qh * 512:(qh + 1) * 512],
                            _mm(kt_pair[rows, kt * 128:(kt + 1) * 128]),
                            _mm(qt_pair[rows, qh * 512:(qh + 1) * 512]),
                            start=True, stop=True,
                        )
                    et = et_pool.tile([128, N], BF16, name=f"et{h}_{kt}", tag="et")
                    nc.scalar.activation(et[:, :], ps[:, :], Exp, scale=SCALE)
                    vcol = vsb[:, kt * 384 + j * 128: kt * 384 + (j + 1) * 128]
                    for qh in range(2):
                        nc.tensor.matmul(
                            avt[half][:, qh * 512:(qh + 1) * 512],
                            _mm(vcol),
                            _mm(et[:, qh * 512:(qh + 1) * 512]),
                            start=(kt == 0), stop=(kt == 7),
                        )
            for c in range(8):
                for half in range(2):
                    h = 2 * j + half
                    rows = slice(64 * half, 64 * half + 64)
                    bc = bc_pool.tile([128, 128], F32, name=f"bc{half}", tag="bc")
                    nc.gpsimd.partition_broadcast(
                        bc[:, :], iSr[0:1, h * N + c * 128: h * N + (c + 1) * 128])
                    nc.vector.tensor_mul(
                        outT[rows, j * N + c * 128: j * N + (c + 1) * 128],
                        avt[half][rows, c * 128:(c + 1) * 128], bc[rows, :])
            for half in range(2):
                h = 2 * j + half
                rows = slice(64 * half, 64 * half + 64)
                bc = bc_pool.tile([128, 128], F32, name=f"bcf{half}", tag="bc")
                nc.gpsimd.partition_broadcast(bc[:, 0:1], cells[0:1, h * 8 + 4: h * 8 + 5])
                nc.gpsimd.partition_broadcast(bc[:, 1:2], cells[0:1, h * 8 + 6: h * 8 + 7])
                v0 = v0T[rows, j: j + 1]
                col0 = outT[rows, j * N: j * N + 1]
                nc.vector.tensor_scalar_mul(bc[rows, 2:3], v0, bc[rows, 1:2])
                nc.vector.scalar_tensor_tensor(col0, col0, bc[rows, 0:1], bc[rows, 2:3],
                                               op0=ALU.mult, op1=ALU.add)

        # emission order: keep ScalarE (exp) continuously fed; v/qkv fill PE
        emit_qkv(0); emit_qkv(3)
        emit_pass1(0)
        emit_qkv(1); emit_qkv(4)
        emit_pass1(1)
        emit_v()
        emit_pass2(0)
        emit_qkv(2); emit_qkv(5)
        emit_pass1(2)
        emit_pass2(1)
        emit_pass2(2)

        # ---- output projection partial: out_part[n, c] over this group's c_in ----
        for nt in range(8):
            ps = ps_pool.tile([128, N], F32, tag="ps")
            for ch in range(2):
                # bank-aligned regions: [0:384] in bank 0, [512:896] in bank 1
                cs = slice(ch * 512, ch * 512 + 384)
                for ktj in range(3):
                    nc.tensor.matmul(
                        ps[:, cs],
                        _mm(outT[:, ktj * N + nt * 128: ktj * N + (nt + 1) * 128]),
                        _mm(wpjT[:, ktj * C + ch * 384: ktj * C + (ch + 1) * 384]),
                        start=(ktj == 0), stop=False,
                    )
                nc.tensor.matmul(ps[:, cs], _mm(ones[0:1, :]),
                                 _mm(bpj[0:1, ch * 384:(ch + 1) * 384]),
                                 start=False, stop=True)
            ot = out_pool.tile([128, C], BF16, tag="outsb")
            nc.vector.tensor_copy(ot[:, 0:384], ps[:, 0:384])
            nc.vector.tensor_copy(ot[:, 384:768], ps[:, 512:896])
            nc.sync.dma_start(out=cc_in[nt * 128:(nt + 1) * 128, :], in_=ot[:, :])

        # ---- pair ReduceScatter of projection partials: core 2b keeps rows
        # 0:512, core 2b+1 rows 512:1024; host concatenates. ----
        nc.gpsimd.collective_compute(
            "ReduceScatter", ALU.add, replica_groups=REPLICA_GROUPS,
            ins=[cc_in[:, :].opt()], outs=[cc_out[:, :].opt()],
        )
        nc.sync.dma_start(out=out_d[:, :], in_=cc_out[:, :])


    nc.compile()
    _split_waits(nc)
    return nc


def _tiled_cols(a, kk):
    """(kk*128, M) -> (128, kk*M): column block k = rows k*128..(k+1)*128."""
    m = a.shape[1]
    return a.reshape(kk, 128, m).transpose(1, 0, 2).reshape(128, kk * m)


def _split_waits(nc):
    """Walrus codegen caps sync-waits at 1 per instruction (2 for
    EventSemaphore). Spill extra waits onto EventSemaphore NOPs inserted
    just before, on the same engine stream."""
    nid = [0]

    def nop_with(engine, waits):
        nid[0] += 1
        nop = mybir.InstEventSemaphore(name=f"WSPILL-{nid[0]}", ins=[], outs=[])
        nop.engine = engine
        nop.sync_info = mybir.SyncInfo(on_wait=list(waits), on_update=[])
        return nop

    for f in nc.m.functions:
        for blk in f.blocks:
            out = []
            changed = False
            for inst in blk.instructions:
                si = inst.sync_info
                waits = list(si.on_wait) if si is not None and si.on_wait else []
                cap = 2 if isinstance(inst, mybir.InstEventSemaphore) else 1
                if len(waits) > cap:
                    spill, keep = waits[:-cap], waits[-cap:]
                    for i in range(0, len(spill), 2):
                        out.append(nop_with(inst.engine, spill[i:i + 2]))
                    inst.sync_info = mybir.SyncInfo(
                        on_wait=keep, on_update=list(si.on_update) if si.on_update else [])
                    changed = True
                out.append(inst)
            if changed:
                blk.instructions = out


def make_in_maps(x, qkv_w, qkv_b, proj_w, proj_b, cls_bias):
    import ml_dtypes
    f = np.float32
    bf = ml_dtypes.bfloat16
    in_maps = []
    for core in range(NCORES):
        b, g = core // 2, core % 2
        hs = g * HPC
        qrows = qkv_w[hs * HD:(hs + HPC) * HD]            # (384, 768)
        krows = qkv_w[C + hs * HD: C + (hs + HPC) * HD]   # (384, 768)
        vrows = qkv_w[2 * C + hs * HD: 2 * C + (hs + HPC) * HD]
        bq = qkv_b[hs * HD:(hs + HPC) * HD]
        bk = qkv_b[C + hs * HD: C + (hs + HPC) * HD]
        bvv = qkv_b[2 * C + hs * HD: 2 * C + (hs + HPC) * HD]

        packed = np.zeros((128, PACKED), f)
        packed[:, OFF_xT:OFF_xT + 6 * N] = _tiled_cols(np.asarray(x[b]).T.astype(f), 6)
        packed[:, OFF_wqkT:OFF_wqkT + 6 * 768] = _tiled_cols(
            np.concatenate([qrows, krows], 0).T.astype(f), 6)
        packed[:, OFF_wvT:OFF_wvT + 6 * 384] = _tiled_cols(vrows.T.astype(f), 6)
        packed[:, OFF_wpjT:OFF_wpjT + 3 * C] = _tiled_cols(
            np.asarray(proj_w).T[hs * HD:(hs + HPC) * HD, :].astype(f), 3)
        packed[0, OFF_ones:OFF_ones + 128] = 1.0
        packed[0, OFF_bv:OFF_bv + HPC * HD] = bvv
        packed[0, OFF_bpj:OFF_bpj + C] = np.asarray(proj_b) * 0.5

        small = np.zeros((128, SPACKED), f)
        small[:, SOFF_ident:SOFF_ident + 128] = np.eye(128, dtype=f)
        small[:, SOFF_bqk:SOFF_bqk + 6] = np.concatenate([bq, bk]).reshape(6, 128).T
        small[:, SOFF_bvc:SOFF_bvc + 3] = np.asarray(bvv).reshape(3, 128).T
        small[0, SOFF_cls:SOFF_cls + HPC] = cls_bias[hs:hs + HPC]
        in_maps.append({"inp": packed.astype(bf), "inps": small})
    return in_maps


_CACHED_NC = None


def _get_nc():
    global _CACHED_NC
    if _CACHED_NC is None:
        _CACHED_NC = build_bass()
    return _CACHED_NC


def run(trace=False, **inputs):
    nc = _get_nc()
    in_maps = make_in_maps(**inputs)
    res = bass_utils.run_bass_kernel_spmd(
        nc, in_maps, core_ids=list(range(NCORES)), trace=trace,
    )
    attn = np.empty((B, H, N, N), np.float32)
    out = np.empty((B, N, C), np.float32)
    for core in range(NCORES):
        b, g = core // 2, core % 2
        attn[b, g * HPC:(g + 1) * HPC] = np.asarray(res.results[core]["attn_out"], dtype=np.float32)
        out[b, g * (N // 2):(g + 1) * (N // 2)] = np.asarray(res.results[core]["out_ext"], dtype=np.float32)
    return (out, attn), res


def kernel(**inputs):
    outputs, _ = run(trace=False, **inputs)
    return outputs
